# revision 29
# baseline (speedup 1.0000x reference)
"""RNN-T decoder kernel for TRN2 (8 cores, T-sharded joint, replicated LSTM).

Layout notes
------------
B=8, T=128, U=64, E=512, H=1024 (8 k-chunks), J=640 (5 j-chunks), OD=1024.
Each core handles T-slice [16c, 16c+16) of the joint; the 2-layer LSTM over U
is computed identically (replicated, all 8 batches) on every core.

Gate permutation: hidden dim is split in 4 quarters (col-tile groups). Group
j's 1024 gate columns are [i_j | f_j | o_j | g_j] (256 each), where x_j acts
on hidden units [256j, 256j+256). Weights/bias/X tensors are host-permuted
to this order.

Gates PSUM tile (128, 1024): group j occupies partitions [32j, 32j+8)
(batch-major), accumulated by 4-way column-packed matmuls (tile_position).

Schedule: software-pipelined wavefronts with layer-1 lagging LAG=18 steps.
Per wavefront the PE runs [T(l0,u-1), R(l0,u), T(l1,v-1), R(l1,v)] back to
back; each layer's activation chain (ACT+DVE) is emitted after all PE work
so neither layer's chain gates the other's stream. X0 is produced in
u-blocks ahead of consumption (block 0 up front, the rest one weight-chunk
per early wavefront); X1 blocks are split in 4-chunk sub-blocks spread over
2 wavefronts with sync/scalar dual-queue fp8 weight loads. h0 history is
kept in both f16 (recurrence) and fp8 (X1 matmuls). The joint runs after
the LSTM with fused broadcast-add/tanh over all jc and f16 weights/output.
"""
import numpy as np
import ml_dtypes

import concourse.bass as bass
import concourse.bacc as bacc
import concourse.mybir as mybir
import concourse.tile as tile

dt = mybir.dt
AF = mybir.ActivationFunctionType

B, T, E, H, J, OD = 8, 128, 512, 1024, 640, 1024
HK = H // 128   # 8 h-chunks
JC = J // 128   # 5 j-chunks
EK = E // 128   # 4 e-chunks
TSH = T // 8    # 16 t per core
NG = 4          # col-tile groups


def bcast_mid(ap, count):
    """(128, N) AP -> (128, count, N) with a 0-step middle dim."""
    return bass.AP(ap.tensor, ap.offset, [ap.ap[0], [0, count], ap.ap[1]])


def build_program(U=64, n_cores=8, with_biases=False,
                  with_out_bias=False):
    nc = bacc.Bacc("TRN2", target_bir_lowering=False, debug=False,
                   num_devices=n_cores)
    f16, f32, f32r, i32 = dt.float16, dt.float32, dt.float32r, dt.int32
    UG = U // 16  # u-blocks of 16
    assert U % 16 == 0

    # ---------------- external inputs ----------------
    embed_d = nc.dram_tensor("embed", [OD, E], f32, kind="ExternalInput")
    yidx_d = nc.dram_tensor("yidx", [128, B * U // 128], i32, kind="ExternalInput")
    wih0_d = nc.dram_tensor("wih0t", [128, EK, 4096], f16, kind="ExternalInput")
    wih1_d = nc.dram_tensor("wih1t", [16, 128, HK, 256], f16, kind="ExternalInput")
    whh0_d = nc.dram_tensor("whh0t", [128, HK, NG, 1024], f16, kind="ExternalInput")
    whh1_d = nc.dram_tensor("whh1t", [128, HK, NG, 1024], f16, kind="ExternalInput")
    inj_d = nc.dram_tensor("inj8", [8, 8], f16, kind="ExternalInput")
    eye128_d = nc.dram_tensor("eye128", [128, 128], f16, kind="ExternalInput")
    wenc_d = nc.dram_tensor("wenct", [128, HK, JC, 128], f16, kind="ExternalInput")
    wdec_d = nc.dram_tensor("wdect", [128, HK, JC, 128], f16, kind="ExternalInput")
    wout_d = nc.dram_tensor("woutt", [128, JC, OD], f16, kind="ExternalInput")
    benc_d = nc.dram_tensor("benc", [128, JC], f32, kind="ExternalInput")
    bout_d = nc.dram_tensor("boutrep", [128, OD], f32, kind="ExternalInput")
    hst_d = nc.dram_tensor("hst16", [128, HK, B * TSH], f16, kind="ExternalInput")
    # per-layer (b_ih + b_hh), gate-permuted, replicated over partitions
    bi0_d = nc.dram_tensor("bihh0", [128, 4096], f16, kind="ExternalInput")
    bi1_d = nc.dram_tensor("bihh1", [128, 4096], f16, kind="ExternalInput")

    out_d = nc.dram_tensor("out", [B * TSH * U, OD], f16, kind="ExternalOutput")

    # ---------------- internal dram ----------------
    x0_d = nc.dram_tensor("X0d", [U, B, 4096], f16)
    x1_d = nc.dram_tensor("X1d", [U, B, 4096], f16)

    LAG = 20  # layer-1 runs LAG wavefronts behind layer-0

    with tile.TileContext(nc) as tc:
        with tc.tile_pool(name="const", bufs=1) as pc:
            # constants (small, urgent loads first on the sync queue)
            eye128_sb = pc.tile([128, 128], f16, tag="eye128")
            nc.sync.dma_start(eye128_sb[:], eye128_d.ap())
            yidx_sb = pc.tile([128, B * U // 128], i32, tag="yidx")
            nc.sync.dma_start(yidx_sb[:], yidx_d.ap())
            inj_sb = pc.tile([8, 8], f16, tag="inj")
            nc.sync.dma_start(inj_sb[:], inj_d.ap())
            hst_sb = pc.tile([128, HK, B * TSH], f16, tag="hst")
            nc.scalar.dma_start(hst_sb[:], hst_d.ap())
            if with_biases:
                bi0_sb = pc.tile([128, 4096], f16, tag="bi0")
                nc.scalar.dma_start(bi0_sb[:], bi0_d.ap())
                bi1_sb = pc.tile([128, 4096], f16, tag="bi1")
                nc.scalar.dma_start(bi1_sb[:], bi1_d.ap())
            # h_dec transposed history, both layers (fp16)
            hdec = [pc.tile([128, HK, U, B], f16, tag=f"hdec{l}",
                            name=f"hdec{l}") for l in range(2)]

            pw_ctx = tc.tile_pool(name="whh", bufs=1)
            pw = pw_ctx.__enter__()
            # recurrent weights (resident for whole LSTM); per-kc chunked
            # loads on the vector queue so early chunks arrive early and
            # don't block the sync queue.
            whh_sb = [pw.tile([128, HK, NG, 1024], f16, tag=f"whh{l}",
                               name=f"whh{l}") for l in range(2)]
            # whh0 loads are emitted after x0_block(0) (same scalar queue)
            # so the startup-critical X0 weight chunks go first; whh1 loads
            # are deferred into the wavefront loop (needed from wavefront
            # LAG on).

            # ---------------- main pools (LSTM + pre phases) --------------
            with (
                tc.tile_pool(name="lstmS", bufs=1) as lS,
                tc.tile_pool(name="lstmPS", bufs=1, space="PSUM") as lP,
            ):
                # ---------------- P1: embedding gather + eys^T ------------
                sc = nc.named_scope("gather"); sc.__enter__()
                NCH = B * U // 128  # row chunks of 128
                eyst = lS.tile([128, EK, B * U], f16, tag="eyst")
                for ch in range(NCH):
                    g32 = lS.tile([128, E], f32, tag="g32", bufs=2)
                    nc.gpsimd.indirect_dma_start(
                        out=g32[:], out_offset=None, in_=embed_d.ap(),
                        in_offset=bass.IndirectOffsetOnAxis(
                            ap=yidx_sb[:, ch:ch + 1], axis=0))
                    g16 = lS.tile([128, E], f16, tag="g16", bufs=2)
                    nc.vector.tensor_copy(g16[:], g32[:])
                    for ec in range(EK):
                        tp = lP.tile([128, 128], f16, tag="tp128", bufs=2)
                        nc.tensor.transpose(
                            tp[:], g16[:, ec * 128:(ec + 1) * 128], eye128_sb[:])
                        nc.vector.tensor_copy(
                            eyst[:, ec, ch * 128:(ch + 1) * 128], tp[:])
                sc.__exit__(None, None, None)

                gate_ps = [lP.tile([128, 1024], f32, tag=f"gates{l}",
                                   name=f"gates{l}") for l in range(2)]
                nc.vector.memset(gate_ps[0][:], 0.0)
                nc.vector.memset(gate_ps[1][:], 0.0)
                czero = [lS.tile([128, 256], f32, tag=f"c{l}", name=f"cz{l}",
                                  bufs=2) for l in range(2)]
                nc.gpsimd.memset(czero[0][:], 0.0)
                nc.gpsimd.memset(czero[1][:], 0.0)
                cprev = [czero[0], czero[1]]
                xsrc = [x0_d, x1_d]

                # ---------------- P2: X0 u-blocks -------------------------
                def x0_block(g):
                    # X0[u-block g] = eys-block @ W_ih0^T, streamed weights
                    for nc_ in range(8):
                        w0c = lS.tile([128, EK, 512], f16, tag="w0c", bufs=2)
                        eng = nc.sync if nc_ % 2 == 0 else nc.scalar
                        eng.dma_start(
                            w0c[:],
                            wih0_d.ap()[:, :, nc_ * 512:(nc_ + 1) * 512])
                        ps = lP.tile([128, 512], f32, tag="xps", bufs=2)
                        for ec in range(EK):
                            nc.tensor.matmul(
                                ps[:],
                                eyst[:, ec, g * 128:(g + 1) * 128],
                                w0c[:, ec, :],
                                start=(ec == 0), stop=(ec == EK - 1))
                        x0c = lS.tile([128, 512], f16, tag="x0c", bufs=3)
                        if with_biases:
                            nc.vector.tensor_add(
                                x0c[:], ps[:],
                                bi0_sb[:, nc_ * 512:(nc_ + 1) * 512])
                        else:
                            nc.vector.tensor_copy(x0c[:], ps[:])
                        nc.sync.dma_start(
                            x0_d.ap()[g * 16:(g + 1) * 16, :,
                                      nc_ * 512:(nc_ + 1) * 512],
                            x0c[:])

                def x0_rest(nc_):
                    # one W_ih0 chunk, X0 for u-blocks 1..3 (loads W once)
                    w0c = lS.tile([128, EK, 512], f16, tag="w0c", bufs=2)
                    eng = nc.sync if nc_ % 2 == 0 else nc.scalar
                    eng.dma_start(
                        w0c[:], wih0_d.ap()[:, :, nc_ * 512:(nc_ + 1) * 512])
                    for g in range(1, UG):
                        ps = lP.tile([128, 512], f32, tag="xps", bufs=2)
                        for ec in range(EK):
                            nc.tensor.matmul(
                                ps[:],
                                eyst[:, ec, g * 128:(g + 1) * 128],
                                w0c[:, ec, :],
                                start=(ec == 0), stop=(ec == EK - 1))
                        x0c = lS.tile([128, 512], f16, tag="x0c", bufs=3)
                        if with_biases:
                            nc.vector.tensor_add(
                                x0c[:], ps[:],
                                bi0_sb[:, nc_ * 512:(nc_ + 1) * 512])
                        else:
                            nc.vector.tensor_copy(x0c[:], ps[:])
                        nc.sync.dma_start(
                            x0_d.ap()[g * 16:(g + 1) * 16, :,
                                      nc_ * 512:(nc_ + 1) * 512],
                            x0c[:])

                # ---------------- P3: LSTM pieces -------------------------
                def lstm_rec(l, u):
                    # inject + recurrent matmuls into gates psum [PE].
                    # inject first: it has no h-dependency, so it can run
                    # during the previous step's activation chain.
                    pg = gate_ps[l]
                    xf = lS.tile([8, 4096], f16, tag="xf", bufs=2)
                    nc.gpsimd.dma_start(xf[:], xsrc[l].ap()[u])
                    for hf in range(2):
                        sl = slice(hf * 512, (hf + 1) * 512)
                        for j in range(NG):
                            nc.tensor.matmul(
                                pg[32 * j:32 * j + 8, sl], inj_sb[:],
                                xf[:, j * 1024 + hf * 512:
                                   j * 1024 + (hf + 1) * 512],
                                tile_position=(0, 32 * j),
                                start=True, stop=(u == 0))
                    if u > 0:
                        for kc in range(HK):
                            for hf in range(2):
                                sl = slice(hf * 512, (hf + 1) * 512)
                                for j in range(NG):
                                    nc.tensor.matmul(
                                        pg[32 * j:32 * j + 8, sl],
                                        hdec[l][:, kc, u - 1, :],
                                        whh_sb[l][:, kc, j,
                                                  hf * 512:(hf + 1) * 512],
                                        tile_position=(0, 32 * j),
                                        start=False, stop=(kc == HK - 1))

                hbuf = [None, None]  # last h tile per layer

                def lstm_chain(l, u, solo=False):
                    # gates -> sigmoid/tanh -> c,h  [ACT + DVE only]
                    # sigmoid split i,f | o so the c-path starts ~0.4us
                    # earlier.
                    pg = gate_ps[l]
                    sig = lS.tile([128, 768], f16, tag=f"sig{l}")
                    nc.scalar.activation(sig[:, 0:512], pg[:, 0:512],
                                         AF.Sigmoid)
                    tg = lS.tile([128, 256], f16, tag=f"tg{l}")
                    nc.scalar.activation(tg[:], pg[:, 768:1024], AF.Tanh)
                    nc.scalar.activation(sig[:, 512:768], pg[:, 512:768],
                                         AF.Sigmoid)
                    cnew = lS.tile([128, 256], f32, tag=f"c{l}", bufs=2)
                    nc.vector.tensor_mul(cnew[:], sig[:, 256:512], cprev[l][:])
                    t1 = lS.tile([128, 256], f32, tag=f"t1{l}")
                    nc.vector.tensor_mul(t1[:], sig[:, 0:256], tg[:])
                    nc.vector.tensor_add(cnew[:], cnew[:], t1[:])
                    cprev[l] = cnew
                    tc_ = lS.tile([128, 256], f16, tag=f"tc{l}")
                    nc.scalar.activation(tc_[:], cnew[:], AF.Tanh)
                    h = lS.tile([128, 256], f16, tag=f"h{l}", bufs=2)
                    nc.vector.tensor_mul(h[:], sig[:, 512:768], tc_[:])
                    hbuf[l] = h

                def lstm_transp(l, u):
                    # h -> hdec[l][:, :, u, :]  [PE transpose + DVE copy]
                    h = hbuf[l]
                    for cb in range(2):
                        tp = lP.tile([128, 128], f16, tag="tp128", bufs=2)
                        nc.tensor.transpose(
                            tp[:], h[:, cb * 128:(cb + 1) * 128],
                            eye128_sb[:])
                        hd = hdec[l][:, 0, u, :]  # (128, B) at kc=0
                        dst = bass.AP(hd.tensor, hd.offset + cb * U * B,
                                      [hd.ap[0], [2 * U * B, NG], [1, B]])
                        src_ap = bass.AP(tp[:].tensor, tp[:].offset,
                                         [tp[:].ap[0], [32, NG], [1, B]])
                        nc.vector.tensor_copy(dst, src_ap)

                def x1_subblock(kb, sb):
                    # 4 of 16 weight chunks of X1 u-block kb; loads alternate
                    # sync/scalar DMA queues to double effective bandwidth
                    hd0 = hdec[0]
                    for nc2 in range(4 * sb, 4 * sb + 4):
                        w1c = lS.tile([128, HK, 256], f16, tag="w1c", bufs=2)
                        eng = nc.sync if nc2 % 2 == 0 else nc.scalar
                        eng.dma_start(w1c[:], wih1_d.ap()[nc2])
                        ps = lP.tile([128, 512], f32, tag="xps", bufs=2)
                        for kc in range(HK):
                            nc.tensor.matmul(
                                ps[:, 0:256],
                                hd0[:, kc, kb * 16:(kb + 1) * 16, :],
                                w1c[:, kc, :],
                                start=(kc == 0), stop=(kc == HK - 1))
                        x1c = lS.tile([128, 256], f16, tag="x1c", bufs=3)
                        if with_biases:
                            nc.vector.tensor_add(
                                x1c[:], ps[:, 0:256],
                                bi1_sb[:, nc2 * 256:(nc2 + 1) * 256])
                        else:
                            nc.vector.tensor_copy(x1c[:], ps[:, 0:256])
                        nc.sync.dma_start(
                            x1_d.ap()[kb * 16:(kb + 1) * 16, :,
                                      nc2 * 256:(nc2 + 1) * 256],
                            x1c[:])

                # ---------------- wavefront schedule ----------------------
                # per wavefront w:
                #   PE:  T(l0,w-1) R(l0,w) T(l1,w-1-LAG) R(l1,w-LAG) [x1blk]
                #   chains emitted after all PE work so neither layer's
                #   ACT/DVE ops gate the other layer's PE stream.
                with nc.named_scope("x0b0"):
                    x0_block(0)
                for kc in range(HK):
                    nc.scalar.dma_start(whh_sb[0][:, kc, :, :],
                                        whh0_d.ap()[:, kc, :, :])
                for w in range(U + LAG + 1):
                    u0, u1 = w, w - LAG
                    with nc.named_scope(f"w{w:02d}"):
                        if 1 <= u0 <= U:
                            lstm_transp(0, u0 - 1)
                        if u0 < U:
                            lstm_rec(0, u0)
                        if 1 <= u1 <= U:
                            lstm_transp(1, u1 - 1)
                        if 0 <= u1 < U:
                            lstm_rec(1, u1)
                        dual = (u0 < U) and (0 <= u1 < U)
                        if u0 < U:
                            lstm_chain(0, u0, solo=not dual)
                        if 0 <= u1 < U:
                            lstm_chain(1, u1, solo=not dual)
                    if 1 <= u0 <= 8:
                        with nc.named_scope(f"x0r{u0 - 1}"):
                            x0_rest(u0 - 1)
                    if u0 == 10:
                        for kc in range(HK):
                            nc.scalar.dma_start(whh_sb[1][:, kc, :, :],
                                                whh1_d.ap()[:, kc, :, :])
                    if 16 <= u0 < 68 and (u0 - 16) % 16 < 4:
                        kb, sb = (u0 - 16) // 16, (u0 - 16) % 16
                        with nc.named_scope(f"x1b{kb}_{sb}"):
                            x1_subblock(kb, sb)

            pw_ctx.__exit__(None, None, None)

            # ---------------- P4/P5: joint ----------------
            sc_j = nc.named_scope("joint"); sc_j.__enter__()
            with (
                tc.tile_pool(name="jS", bufs=1) as jS,
                tc.tile_pool(name="jPS", bufs=1, space="PSUM") as jP,
            ):
                wenc_sb = jS.tile([128, HK, JC, 128], f16, tag="wenc")
                nc.sync.dma_start(wenc_sb[:], wenc_d.ap())
                wdec_sb = jS.tile([128, HK, JC, 128], f16, tag="wdec")
                nc.sync.dma_start(wdec_sb[:], wdec_d.ap())
                wout_sb = jS.tile([128, JC, OD], f16, tag="woutr")
                nc.sync.dma_start(wout_sb[:], wout_d.ap())
                benc_sb = jS.tile([128, JC], f32, tag="bencs")
                nc.sync.dma_start(benc_sb[:], benc_d.ap())
                if with_out_bias:
                    bout_sb = jS.tile([128, OD], f32, tag="bouts")
                    nc.sync.dma_start(bout_sb[:], bout_d.ap())

                # ze (J, b*tl) and zd (J, b, u)
                sc_z = nc.named_scope("zedzd"); sc_z.__enter__()
                ze_sb = jS.tile([128, JC, B * TSH], f16, tag="ze")
                for jc in range(JC):
                    zp = jP.tile([128, B * TSH], f32, tag="zeps", bufs=2)
                    for ec in range(HK):
                        nc.tensor.matmul(zp[:], wenc_sb[:, ec, jc, :],
                                         hst_sb[:, ec, :],
                                         start=(ec == 0), stop=(ec == HK - 1))
                    nc.scalar.activation(ze_sb[:, jc, :], zp[:], AF.Identity,
                                         bias=benc_sb[:, jc:jc + 1])
                zd_sb = jS.tile([128, JC, U, B], f16, tag="zd")
                for jc in range(JC):
                    zp = jP.tile([128, U * B], f32, tag="zdps", bufs=2)
                    for kc in range(HK):
                        nc.tensor.matmul(
                            zp[:], wdec_sb[:, kc, jc, :],
                            hdec[1][:, kc, :, :].rearrange("p u b -> p (u b)"),
                            start=(kc == 0), stop=(kc == HK - 1))
                    nc.vector.tensor_copy(
                        zd_sb[:, jc, :, :].rearrange("p u b -> p (u b)"), zp[:])

                sc_z.__exit__(None, None, None)
                sc_b = nc.named_scope("jblk"); sc_b.__enter__()
                # joint blocks: 128 rows = 2 (b,tl) pairs x U
                n_pairs = B * TSH
                rows_per_pair = U
                ppb = 128 // rows_per_pair  # pairs per block
                BTSH = B * TSH
                for blk in range(n_pairs // ppb):
                    pr0 = blk * ppb
                    b = pr0 // TSH
                    zjt = jS.tile([128, JC, 128], f16, tag="zjt", bufs=2)
                    zj = jS.tile([128, JC, 128], f16, tag="zj", bufs=2)
                    # fused over all jc: out[p, jc, a, u] = ze[p, jc, pr0+a]
                    #                                     + zd[p, jc, u, b]
                    zjt_ap = zjt[:, :, :].rearrange(
                        "p jc (a u) -> p jc a u", a=ppb)
                    zea = ze_sb[:, 0, 0]
                    ze_bc = bass.AP(zea.tensor, zea.offset + pr0,
                                    [zea.ap[0], [BTSH, JC], [1, ppb], [0, U]])
                    zda = zd_sb[:, 0, 0, 0]
                    zd_bc = bass.AP(zda.tensor, zda.offset + b,
                                    [zda.ap[0], [U * B, JC], [0, ppb], [B, U]])
                    nc.vector.tensor_tensor(
                        zjt_ap, ze_bc, zd_bc, op=mybir.AluOpType.add)
                    nc.scalar.activation(
                        zj[:, :, :].rearrange("p jc m -> p (jc m)"),
                        zjt[:, :, :].rearrange("p jc m -> p (jc m)"), AF.Tanh)
                    ops_ = jP.tile([128, OD], f32, tag="outps", bufs=2)
                    for n2 in range(2):
                        for jc in range(JC):
                            nc.tensor.matmul(
                                ops_[:, n2 * 512:(n2 + 1) * 512],
                                zj[:, jc, :],
                                wout_sb[:, jc, n2 * 512:(n2 + 1) * 512],
                                start=(jc == 0), stop=(jc == JC - 1))
                    osb = jS.tile([128, OD], f16, tag="osb", bufs=3)
                    if with_out_bias:
                        nc.vector.tensor_add(osb[:], ops_[:], bout_sb[:])
                    else:
                        # split the psum->sbuf evacuation across DVE + ACT
                        # so neither engine bottlenecks the block pipeline
                        nc.vector.tensor_copy(osb[:, 0:512], ops_[:, 0:512])
                        nc.scalar.activation(osb[:, 512:1024],
                                             ops_[:, 512:1024], AF.Identity)
                    eng = nc.sync if blk % 2 == 0 else nc.scalar
                    eng.dma_start(
                        out_d.ap()[blk * 128:(blk + 1) * 128, :], osb[:])
                sc_b.__exit__(None, None, None)
            sc_j.__exit__(None, None, None)

    nc.compile()
    return nc


# ---------------- host-side prep ----------------

def gate_perm():
    """perm[j*1024 + s] -> row index in torch (i,f,g,o) 4H gate layout,
    with group-local order [i|f|o|g]."""
    perm = np.zeros(4 * H, dtype=np.int64)
    for j in range(NG):
        base = j * 1024
        hid = np.arange(256) + j * 256
        perm[base + 0:base + 256] = 0 * H + hid      # i
        perm[base + 256:base + 512] = 1 * H + hid    # f
        perm[base + 512:base + 768] = 3 * H + hid    # o
        perm[base + 768:base + 1024] = 2 * H + hid   # g
    return perm


def prep_inputs(hs_pad, ys_in_pad, embed, W_ih0, W_hh0, b_ih0, b_hh0,
                W_ih1, W_hh1, b_ih1, b_hh1, W_enc, b_enc, W_dec, W_out, b_out,
                U=64, n_cores=8):
    f16 = ml_dtypes.float16 if not hasattr(np, "float16") else np.float16
    perm = gate_perm()

    def wiht(W, KD, KC):  # (4H, KD) -> (128, KC, 4096) fp16, permuted gates
        Wp = W[perm]                      # (4096, KD)
        return np.ascontiguousarray(
            Wp.T.reshape(KC, 128, 4096).transpose(1, 0, 2)).astype(np.float16)

    def whht(W):  # (4H, H) -> (128, HK, NG, 1024) fp16
        Wp = W[perm]                      # (4096, 1024) rows=permuted gates
        # [p, kc, j, n] = Wp[j*1024+n, kc*128+p]
        a = Wp.T.reshape(HK, 128, NG, 1024).transpose(1, 0, 2, 3)
        return np.ascontiguousarray(a).astype(np.float16)

    ins = {}
    ins["embed"] = np.asarray(embed, np.float32)
    ys = np.asarray(ys_in_pad).astype(np.int32)   # (B, U)
    NCH = B * U // 128
    yy = np.zeros((128, NCH), np.int32)
    for ch in range(NCH):
        p = np.arange(128)
        yy[:, ch] = ys[p % 8, ch * 16 + p // 8]
    ins["yidx"] = yy
    ins["wih0t"] = wiht(W_ih0, E, EK)
    w1 = wiht(W_ih1, H, HK)  # (128, HK, 4096)
    ins["wih1t"] = np.ascontiguousarray(
        w1.reshape(128, HK, 16, 256).transpose(2, 0, 1, 3))
    ins["whh0t"] = whht(W_hh0)
    ins["whh1t"] = whht(W_hh1)
    ins["inj8"] = np.eye(8, dtype=np.float16)
    ins["eye128"] = np.eye(128, dtype=np.float16)
    # [p, ec, jc, m] = W[jc*128+m, ec*128+p]
    def wjt(W, KC):
        a = W.T.reshape(KC, 128, JC, 128).transpose(1, 0, 2, 3)
        return np.ascontiguousarray(a).astype(np.float16)
    ins["wenct"] = wjt(W_enc, HK)
    ins["wdect"] = wjt(W_dec, HK)
    # [p, jc, od] = W_out[od, jc*128+p]
    ins["woutt"] = np.ascontiguousarray(
        W_out.T.reshape(JC, 128, OD).transpose(1, 0, 2)).astype(np.float16)
    ins["benc"] = np.ascontiguousarray(
        b_enc.reshape(JC, 128).T).astype(np.float32)
    ins["boutrep"] = np.tile(np.asarray(b_out, np.float32)[None, :], (128, 1))
    ins["bihh0"] = np.tile(((b_ih0 + b_hh0)[perm]).astype(np.float16)[None, :],
                           (128, 1))
    ins["bihh1"] = np.tile(((b_ih1 + b_hh1)[perm]).astype(np.float16)[None, :],
                           (128, 1))

    maps = []
    for c in range(n_cores):
        m = dict(ins)
        # [p, ec, r] = hs[b, TSH*c + tl, ec*128+p], r = b*TSH+tl
        sl = np.asarray(hs_pad[:, TSH * c:TSH * (c + 1), :], np.float32)
        a = sl.reshape(B * TSH, HK, 128).transpose(2, 1, 0)
        m["hst16"] = np.ascontiguousarray(a).astype(np.float16)
        maps.append(m)
    return maps


def gather_output(results):
    outs = [np.asarray(r["out"], np.float32).reshape(B, TSH, -1, OD)
            for r in results]
    return np.concatenate(outs, axis=1)


# ---------------- entry point ----------------
import sys as _sys
import types as _types

# Recreate the missing antenv.axon_hooks so trace=True works under axon
# (used only when BASS_TRACE=1 is set by a profiling harness).
if "antenv.axon_hooks" not in _sys.modules:
    _m = _types.ModuleType("antenv.axon_hooks")

    def _get_hook():
        try:
            from trn_agent_boot.trn_boot import _ntff_profile_via_ctypes
            return _ntff_profile_via_ctypes("/opt/axon/libaxon_pjrt.so")
        except Exception:
            return None
    _m.get_axon_ntff_profile_hook = _get_hook
    _sys.modules["antenv.axon_hooks"] = _m

_NC = None
last_results = None


def kernel(**inputs):
    """Full-input RNN-T decoder: returns (B, T, U, ODIM) float32."""
    global _NC, last_results
    from concourse.bass_utils import run_bass_kernel_spmd
    U = int(np.asarray(inputs["ys_in_pad"]).shape[1])
    wb = any(float(np.abs(np.asarray(inputs[k])).max()) != 0.0
             for k in ("b_ih0", "b_hh0", "b_ih1", "b_hh1"))
    wob = float(np.abs(np.asarray(inputs["b_out"])).max()) != 0.0
    if _NC is None:
        _NC = build_program(U=U, n_cores=8, with_biases=wb, with_out_bias=wob)
    maps = prep_inputs(**inputs, U=U)
    res = run_bass_kernel_spmd(_NC, maps, core_ids=list(range(8)))
    last_results = res
    return gather_output(res.results)



# revision 31
# speedup vs baseline: 1.1860x; 1.1860x over previous
"""RNN-T decoder kernel for TRN2 (8 cores, T-sharded joint, replicated LSTM).

Layout notes
------------
B=8, T=128, U=64, E=512, H=1024 (8 k-chunks), J=640 (5 j-chunks), OD=1024.
Each core handles T-slice [16c, 16c+16) of the joint; the 2-layer LSTM over U
is computed identically (replicated, all 8 batches) on every core.

Gate permutation: hidden dim is split in 4 quarters (col-tile groups). Group
j's 1024 gate columns are [i_j | f_j | o_j | g_j] (256 each), where x_j acts
on hidden units [256j, 256j+256). Weights/bias/X tensors are host-permuted
to this order.

Gates PSUM tile (128, 1024): group j occupies partitions [32j, 32j+8)
(batch-major), accumulated by 4-way column-packed matmuls (tile_position).

Schedule: software-pipelined wavefronts with layer-1 lagging LAG=18 steps.
Per wavefront the PE runs [T(l0,u-1), R(l0,u), T(l1,v-1), R(l1,v)] back to
back; each layer's activation chain (ACT+DVE) is emitted after all PE work
so neither layer's chain gates the other's stream. X0 is produced in
u-blocks ahead of consumption (block 0 up front, the rest one weight-chunk
per early wavefront); X1 blocks are split in 4-chunk sub-blocks spread over
2 wavefronts with sync/scalar dual-queue fp8 weight loads. h0 history is
kept in both f16 (recurrence) and fp8 (X1 matmuls). The joint runs after
the LSTM with fused broadcast-add/tanh over all jc and f16 weights/output.
"""
import numpy as np
import ml_dtypes

import concourse.bass as bass
import concourse.bacc as bacc
import concourse.mybir as mybir
import concourse.tile as tile

dt = mybir.dt
AF = mybir.ActivationFunctionType

B, T, E, H, J, OD = 8, 128, 512, 1024, 640, 1024
HK = H // 128   # 8 h-chunks
JC = J // 128   # 5 j-chunks
EK = E // 128   # 4 e-chunks
TSH = T // 8    # 16 t per core
NG = 4          # col-tile groups


def bcast_mid(ap, count):
    """(128, N) AP -> (128, count, N) with a 0-step middle dim."""
    return bass.AP(ap.tensor, ap.offset, [ap.ap[0], [0, count], ap.ap[1]])


def build_program(U=64, n_cores=8, with_biases=False,
                  with_out_bias=False):
    nc = bacc.Bacc("TRN2", target_bir_lowering=False, debug=False,
                   num_devices=n_cores)
    f16, f32, f32r, i32 = dt.float16, dt.float32, dt.float32r, dt.int32
    UG = U // 16  # u-blocks of 16
    assert U % 16 == 0

    # ---------------- external inputs ----------------
    embed_d = nc.dram_tensor("embed", [OD, E], f32, kind="ExternalInput")
    yidx_d = nc.dram_tensor("yidx", [128, B * U // 128], i32, kind="ExternalInput")
    wih0_d = nc.dram_tensor("wih0t", [128, EK, 4096], f16, kind="ExternalInput")
    wih1_d = nc.dram_tensor("wih1t", [16, 128, HK, 256], f16, kind="ExternalInput")
    whh0_d = nc.dram_tensor("whh0t", [128, HK, NG, 1024], f16, kind="ExternalInput")
    whh1_d = nc.dram_tensor("whh1t", [128, HK, NG, 1024], f16, kind="ExternalInput")
    inj_d = nc.dram_tensor("inj8", [8, 8], f16, kind="ExternalInput")
    eye128_d = nc.dram_tensor("eye128", [128, 128], f16, kind="ExternalInput")
    wenc_d = nc.dram_tensor("wenct", [128, HK, JC, 128], f16, kind="ExternalInput")
    wdec_d = nc.dram_tensor("wdect", [128, HK, JC, 128], f16, kind="ExternalInput")
    wout_d = nc.dram_tensor("woutt", [128, JC, OD], f16, kind="ExternalInput")
    benc_d = nc.dram_tensor("benc", [128, JC], f32, kind="ExternalInput")
    bout_d = nc.dram_tensor("boutrep", [128, OD], f32, kind="ExternalInput")
    hst_d = nc.dram_tensor("hst16", [128, HK, B * TSH], f16, kind="ExternalInput")
    # per-layer (b_ih + b_hh), gate-permuted, replicated over partitions
    bi0_d = nc.dram_tensor("bihh0", [128, 4096], f16, kind="ExternalInput")
    bi1_d = nc.dram_tensor("bihh1", [128, 4096], f16, kind="ExternalInput")

    out_d = nc.dram_tensor("out", [B * TSH * U, OD], f16, kind="ExternalOutput")

    # ---------------- internal dram ----------------
    x0_d = nc.dram_tensor("X0d", [U, B, 4096], f16)
    x1_d = nc.dram_tensor("X1d", [U, B, 4096], f16)

    LAG = 20  # layer-1 runs LAG wavefronts behind layer-0

    with tile.TileContext(nc) as tc:
        with tc.tile_pool(name="const", bufs=1) as pc:
            # constants (small, urgent loads first on the sync queue)
            eye128_sb = pc.tile([128, 128], f16, tag="eye128")
            nc.sync.dma_start(eye128_sb[:], eye128_d.ap())
            yidx_sb = pc.tile([128, B * U // 128], i32, tag="yidx")
            nc.sync.dma_start(yidx_sb[:], yidx_d.ap())
            inj_sb = pc.tile([8, 8], f16, tag="inj")
            nc.sync.dma_start(inj_sb[:], inj_d.ap())
            hst_sb = pc.tile([128, HK, B * TSH], f16, tag="hst")
            nc.scalar.dma_start(hst_sb[:], hst_d.ap())
            if with_biases:
                bi0_sb = pc.tile([128, 4096], f16, tag="bi0")
                nc.scalar.dma_start(bi0_sb[:], bi0_d.ap())
                bi1_sb = pc.tile([128, 4096], f16, tag="bi1")
                nc.scalar.dma_start(bi1_sb[:], bi1_d.ap())
            # h_dec transposed history, both layers (fp16)
            hdec = [pc.tile([128, HK, U, B], f16, tag=f"hdec{l}",
                            name=f"hdec{l}") for l in range(2)]

            pw_ctx = tc.tile_pool(name="whh", bufs=1)
            pw = pw_ctx.__enter__()
            # recurrent weights (resident for whole LSTM); per-kc chunked
            # loads on the vector queue so early chunks arrive early and
            # don't block the sync queue.
            whh_sb = [pw.tile([128, HK, NG, 1024], f16, tag=f"whh{l}",
                               name=f"whh{l}") for l in range(2)]
            # whh0 loads are emitted after x0_block(0) (same scalar queue)
            # so the startup-critical X0 weight chunks go first; whh1 loads
            # are deferred into the wavefront loop (needed from wavefront
            # LAG on).

            # ---------------- main pools (LSTM + pre phases) --------------
            with (
                tc.tile_pool(name="lstmS", bufs=1) as lS,
                tc.tile_pool(name="lstmPS", bufs=1, space="PSUM") as lP,
            ):
                # ---------------- P1: embedding gather + eys^T ------------
                sc = nc.named_scope("gather"); sc.__enter__()
                NCH = B * U // 128  # row chunks of 128
                eyst = lS.tile([128, EK, B * U], f16, tag="eyst")
                for ch in range(NCH):
                    g32 = lS.tile([128, E], f32, tag="g32", bufs=1)
                    nc.gpsimd.indirect_dma_start(
                        out=g32[:], out_offset=None, in_=embed_d.ap(),
                        in_offset=bass.IndirectOffsetOnAxis(
                            ap=yidx_sb[:, ch:ch + 1], axis=0))
                    g16 = lS.tile([128, E], f16, tag="g16", bufs=1)
                    nc.vector.tensor_copy(g16[:], g32[:])
                    for ec in range(EK):
                        tp = lP.tile([128, 128], f16, tag="tp128", bufs=2)
                        nc.tensor.transpose(
                            tp[:], g16[:, ec * 128:(ec + 1) * 128], eye128_sb[:])
                        nc.vector.tensor_copy(
                            eyst[:, ec, ch * 128:(ch + 1) * 128], tp[:])
                sc.__exit__(None, None, None)

                gate_ps = [lP.tile([128, 1024], f32, tag=f"gates{l}",
                                   name=f"gates{l}") for l in range(2)]
                nc.vector.memset(gate_ps[0][:], 0.0)
                nc.vector.memset(gate_ps[1][:], 0.0)
                czero = [lS.tile([128, 256], f32, tag=f"c{l}", name=f"cz{l}",
                                  bufs=2) for l in range(2)]
                nc.gpsimd.memset(czero[0][:], 0.0)
                nc.gpsimd.memset(czero[1][:], 0.0)
                cprev = [czero[0], czero[1]]
                xsrc = [x0_d, x1_d]

                # ---------------- P2: X0 u-blocks -------------------------
                def x0_block(g):
                    # X0[u-block g] = eys-block @ W_ih0^T, streamed weights
                    for nc_ in range(8):
                        w0c = lS.tile([128, EK, 512], f16, tag="w0c", bufs=2)
                        eng = nc.sync if nc_ % 2 == 0 else nc.scalar
                        eng.dma_start(
                            w0c[:],
                            wih0_d.ap()[:, :, nc_ * 512:(nc_ + 1) * 512])
                        ps = lP.tile([128, 512], f32, tag="xps", bufs=2)
                        for ec in range(EK):
                            nc.tensor.matmul(
                                ps[:],
                                eyst[:, ec, g * 128:(g + 1) * 128],
                                w0c[:, ec, :],
                                start=(ec == 0), stop=(ec == EK - 1))
                        x0c = lS.tile([128, 512], f16, tag="x0c", bufs=2)
                        if with_biases:
                            nc.vector.tensor_add(
                                x0c[:], ps[:],
                                bi0_sb[:, nc_ * 512:(nc_ + 1) * 512])
                        else:
                            nc.vector.tensor_copy(x0c[:], ps[:])
                        nc.sync.dma_start(
                            x0_d.ap()[g * 16:(g + 1) * 16, :,
                                      nc_ * 512:(nc_ + 1) * 512],
                            x0c[:])

                def x0_rest(nc_):
                    # one W_ih0 chunk, X0 for u-blocks 1..3 (loads W once)
                    w0c = lS.tile([128, EK, 512], f16, tag="w0c", bufs=2)
                    eng = nc.sync if nc_ % 2 == 0 else nc.scalar
                    eng.dma_start(
                        w0c[:], wih0_d.ap()[:, :, nc_ * 512:(nc_ + 1) * 512])
                    for g in range(1, UG):
                        ps = lP.tile([128, 512], f32, tag="xps", bufs=2)
                        for ec in range(EK):
                            nc.tensor.matmul(
                                ps[:],
                                eyst[:, ec, g * 128:(g + 1) * 128],
                                w0c[:, ec, :],
                                start=(ec == 0), stop=(ec == EK - 1))
                        x0c = lS.tile([128, 512], f16, tag="x0c", bufs=2)
                        if with_biases:
                            nc.vector.tensor_add(
                                x0c[:], ps[:],
                                bi0_sb[:, nc_ * 512:(nc_ + 1) * 512])
                        else:
                            nc.vector.tensor_copy(x0c[:], ps[:])
                        nc.sync.dma_start(
                            x0_d.ap()[g * 16:(g + 1) * 16, :,
                                      nc_ * 512:(nc_ + 1) * 512],
                            x0c[:])

                # ---------------- P3: LSTM pieces -------------------------
                def lstm_rec(l, u):
                    # inject + recurrent matmuls into gates psum [PE].
                    # inject first: it has no h-dependency, so it can run
                    # during the previous step's activation chain.
                    pg = gate_ps[l]
                    xf = lS.tile([8, 4096], f16, tag="xf", bufs=2)
                    nc.gpsimd.dma_start(xf[:], xsrc[l].ap()[u])
                    for hf in range(2):
                        sl = slice(hf * 512, (hf + 1) * 512)
                        for j in range(NG):
                            nc.tensor.matmul(
                                pg[32 * j:32 * j + 8, sl], inj_sb[:],
                                xf[:, j * 1024 + hf * 512:
                                   j * 1024 + (hf + 1) * 512],
                                tile_position=(0, 32 * j),
                                start=True, stop=(u == 0))
                    if u > 0:
                        for kc in range(HK):
                            for hf in range(2):
                                sl = slice(hf * 512, (hf + 1) * 512)
                                for j in range(NG):
                                    nc.tensor.matmul(
                                        pg[32 * j:32 * j + 8, sl],
                                        hdec[l][:, kc, u - 1, :],
                                        whh_sb[l][:, kc, j,
                                                  hf * 512:(hf + 1) * 512],
                                        tile_position=(0, 32 * j),
                                        start=False, stop=(kc == HK - 1))

                hbuf = [None, None]  # last h tile per layer

                def lstm_chain(l, u, solo=False):
                    # gates -> sigmoid/tanh -> c,h  [ACT + DVE only]
                    # sigmoid split i,f | o so the c-path starts ~0.4us
                    # earlier.
                    pg = gate_ps[l]
                    sig = lS.tile([128, 768], f16, tag=f"sig{l}")
                    nc.scalar.activation(sig[:, 0:512], pg[:, 0:512],
                                         AF.Sigmoid)
                    tg = lS.tile([128, 256], f16, tag=f"tg{l}")
                    nc.scalar.activation(tg[:], pg[:, 768:1024], AF.Tanh)
                    nc.scalar.activation(sig[:, 512:768], pg[:, 512:768],
                                         AF.Sigmoid)
                    cnew = lS.tile([128, 256], f32, tag=f"c{l}", bufs=2)
                    nc.vector.tensor_mul(cnew[:], sig[:, 256:512], cprev[l][:])
                    t1 = lS.tile([128, 256], f32, tag=f"t1{l}")
                    nc.vector.tensor_mul(t1[:], sig[:, 0:256], tg[:])
                    nc.vector.tensor_add(cnew[:], cnew[:], t1[:])
                    cprev[l] = cnew
                    tc_ = lS.tile([128, 256], f16, tag=f"tc{l}")
                    nc.scalar.activation(tc_[:], cnew[:], AF.Tanh)
                    h = lS.tile([128, 256], f16, tag=f"h{l}", bufs=2)
                    nc.vector.tensor_mul(h[:], sig[:, 512:768], tc_[:])
                    hbuf[l] = h

                def lstm_transp(l, u):
                    # h -> hdec[l][:, :, u, :]  [PE transpose + DVE copy]
                    h = hbuf[l]
                    for cb in range(2):
                        tp = lP.tile([128, 128], f16, tag="tp128", bufs=2)
                        nc.tensor.transpose(
                            tp[:], h[:, cb * 128:(cb + 1) * 128],
                            eye128_sb[:])
                        hd = hdec[l][:, 0, u, :]  # (128, B) at kc=0
                        dst = bass.AP(hd.tensor, hd.offset + cb * U * B,
                                      [hd.ap[0], [2 * U * B, NG], [1, B]])
                        src_ap = bass.AP(tp[:].tensor, tp[:].offset,
                                         [tp[:].ap[0], [32, NG], [1, B]])
                        nc.vector.tensor_copy(dst, src_ap)

                def x1_subblock(kb, sb):
                    # 4 of 16 weight chunks of X1 u-block kb; loads alternate
                    # sync/scalar DMA queues to double effective bandwidth
                    hd0 = hdec[0]
                    for nc2 in range(4 * sb, 4 * sb + 4):
                        w1c = lS.tile([128, HK, 256], f16, tag="w1c", bufs=2)
                        eng = nc.sync if nc2 % 2 == 0 else nc.scalar
                        eng.dma_start(w1c[:], wih1_d.ap()[nc2])
                        ps = lP.tile([128, 512], f32, tag="xps", bufs=2)
                        for kc in range(HK):
                            nc.tensor.matmul(
                                ps[:, 0:256],
                                hd0[:, kc, kb * 16:(kb + 1) * 16, :],
                                w1c[:, kc, :],
                                start=(kc == 0), stop=(kc == HK - 1))
                        x1c = lS.tile([128, 256], f16, tag="x1c", bufs=3)
                        if with_biases:
                            nc.vector.tensor_add(
                                x1c[:], ps[:, 0:256],
                                bi1_sb[:, nc2 * 256:(nc2 + 1) * 256])
                        else:
                            nc.vector.tensor_copy(x1c[:], ps[:, 0:256])
                        nc.sync.dma_start(
                            x1_d.ap()[kb * 16:(kb + 1) * 16, :,
                                      nc2 * 256:(nc2 + 1) * 256],
                            x1c[:])

                # early ze: encoder projection needs only hst + wenc;
                # computed during the l1-solo tail to fill PE idle there
                benc_sb = lS.tile([128, JC], f32, tag="bencs")
                ze_sb = lS.tile([128, JC, B * TSH], f16, tag="ze")

                def ze_compute():
                    nc.sync.dma_start(benc_sb[:], benc_d.ap())
                    for jc in range(JC):
                        wencc = lS.tile([128, HK, 128], f16, tag="wencc",
                                        bufs=2)
                        eng = nc.sync if jc % 2 == 0 else nc.scalar
                        eng.dma_start(wencc[:], wenc_d.ap()[:, :, jc, :])
                        zp = lP.tile([128, 512], f32, tag="xps", bufs=2)
                        for ec in range(HK):
                            nc.tensor.matmul(zp[:, 0:128],
                                             wencc[:, ec, :],
                                             hst_sb[:, ec, :],
                                             start=(ec == 0),
                                             stop=(ec == HK - 1))
                        nc.scalar.activation(ze_sb[:, jc, :], zp[:, 0:128],
                                             AF.Identity,
                                             bias=benc_sb[:, jc:jc + 1])

                # ---------------- wavefront schedule ----------------------
                # per wavefront w:
                #   PE:  T(l0,w-1) R(l0,w) T(l1,w-1-LAG) R(l1,w-LAG) [x1blk]
                #   chains emitted after all PE work so neither layer's
                #   ACT/DVE ops gate the other layer's PE stream.
                with nc.named_scope("x0b0"):
                    x0_block(0)
                for kc in range(HK):
                    nc.scalar.dma_start(whh_sb[0][:, kc, :, :],
                                        whh0_d.ap()[:, kc, :, :])
                for w in range(U + LAG + 1):
                    u0, u1 = w, w - LAG
                    with nc.named_scope(f"w{w:02d}"):
                        if 1 <= u0 <= U:
                            lstm_transp(0, u0 - 1)
                        if u0 < U:
                            lstm_rec(0, u0)
                        if 1 <= u1 <= U:
                            lstm_transp(1, u1 - 1)
                        if 0 <= u1 < U:
                            lstm_rec(1, u1)
                        dual = (u0 < U) and (0 <= u1 < U)
                        if u0 < U:
                            lstm_chain(0, u0, solo=not dual)
                        if 0 <= u1 < U:
                            lstm_chain(1, u1, solo=not dual)
                    if 1 <= u0 <= 8:
                        with nc.named_scope(f"x0r{u0 - 1}"):
                            x0_rest(u0 - 1)
                    if u0 == 10:
                        for kc in range(HK):
                            nc.scalar.dma_start(whh_sb[1][:, kc, :, :],
                                                whh1_d.ap()[:, kc, :, :])
                    if u0 == U + 2:
                        with nc.named_scope("ze"):
                            ze_compute()
                    if 16 <= u0 < 68 and (u0 - 16) % 16 < 4:
                        kb, sb = (u0 - 16) // 16, (u0 - 16) % 16
                        with nc.named_scope(f"x1b{kb}_{sb}"):
                            x1_subblock(kb, sb)

            pw_ctx.__exit__(None, None, None)

            # ---------------- P4/P5: joint ----------------
            sc_j = nc.named_scope("joint"); sc_j.__enter__()
            with (
                tc.tile_pool(name="jS", bufs=1) as jS,
                tc.tile_pool(name="jPS", bufs=1, space="PSUM") as jP,
            ):
                wdec_sb = jS.tile([128, HK, JC, 128], f16, tag="wdec")
                nc.sync.dma_start(wdec_sb[:], wdec_d.ap())
                wout_sb = jS.tile([128, JC, OD], f16, tag="woutr")
                nc.sync.dma_start(wout_sb[:], wout_d.ap())
                if with_out_bias:
                    bout_sb = jS.tile([128, OD], f32, tag="bouts")
                    nc.sync.dma_start(bout_sb[:], bout_d.ap())

                # ze (J, b*tl) and zd (J, b, u)
                sc_z = nc.named_scope("zedzd"); sc_z.__enter__()
                zd_sb = jS.tile([128, JC, U, B], f16, tag="zd")
                for jc in range(JC):
                    zp = jP.tile([128, U * B], f32, tag="zdps", bufs=2)
                    for kc in range(HK):
                        nc.tensor.matmul(
                            zp[:], wdec_sb[:, kc, jc, :],
                            hdec[1][:, kc, :, :].rearrange("p u b -> p (u b)"),
                            start=(kc == 0), stop=(kc == HK - 1))
                    nc.vector.tensor_copy(
                        zd_sb[:, jc, :, :].rearrange("p u b -> p (u b)"), zp[:])

                sc_z.__exit__(None, None, None)
                sc_b = nc.named_scope("jblk"); sc_b.__enter__()
                # joint blocks: 128 rows = 2 (b,tl) pairs x U
                n_pairs = B * TSH
                rows_per_pair = U
                ppb = 128 // rows_per_pair  # pairs per block
                BTSH = B * TSH
                for blk in range(n_pairs // ppb):
                    pr0 = blk * ppb
                    b = pr0 // TSH
                    zjt = jS.tile([128, JC, 128], f16, tag="zjt", bufs=2)
                    zj = jS.tile([128, JC, 128], f16, tag="zj", bufs=2)
                    # fused over all jc: out[p, jc, a, u] = ze[p, jc, pr0+a]
                    #                                     + zd[p, jc, u, b]
                    zjt_ap = zjt[:, :, :].rearrange(
                        "p jc (a u) -> p jc a u", a=ppb)
                    zea = ze_sb[:, 0, 0]
                    ze_bc = bass.AP(zea.tensor, zea.offset + pr0,
                                    [zea.ap[0], [BTSH, JC], [1, ppb], [0, U]])
                    zda = zd_sb[:, 0, 0, 0]
                    zd_bc = bass.AP(zda.tensor, zda.offset + b,
                                    [zda.ap[0], [U * B, JC], [0, ppb], [B, U]])
                    nc.vector.tensor_tensor(
                        zjt_ap, ze_bc, zd_bc, op=mybir.AluOpType.add)
                    nc.scalar.activation(
                        zj[:, :, :].rearrange("p jc m -> p (jc m)"),
                        zjt[:, :, :].rearrange("p jc m -> p (jc m)"), AF.Tanh)
                    ops_ = jP.tile([128, OD], f32, tag="outps", bufs=2)
                    for n2 in range(2):
                        for jc in range(JC):
                            nc.tensor.matmul(
                                ops_[:, n2 * 512:(n2 + 1) * 512],
                                zj[:, jc, :],
                                wout_sb[:, jc, n2 * 512:(n2 + 1) * 512],
                                start=(jc == 0), stop=(jc == JC - 1))
                    osb = jS.tile([128, OD], f16, tag="osb", bufs=3)
                    if with_out_bias:
                        nc.vector.tensor_add(osb[:], ops_[:], bout_sb[:])
                    else:
                        # split the psum->sbuf evacuation across DVE + ACT
                        # so neither engine bottlenecks the block pipeline
                        nc.vector.tensor_copy(osb[:, 0:512], ops_[:, 0:512])
                        nc.scalar.activation(osb[:, 512:1024],
                                             ops_[:, 512:1024], AF.Identity)
                    eng = nc.sync if blk % 2 == 0 else nc.scalar
                    eng.dma_start(
                        out_d.ap()[blk * 128:(blk + 1) * 128, :], osb[:])
                sc_b.__exit__(None, None, None)
            sc_j.__exit__(None, None, None)

    nc.compile()
    return nc


# ---------------- host-side prep ----------------

def gate_perm():
    """perm[j*1024 + s] -> row index in torch (i,f,g,o) 4H gate layout,
    with group-local order [i|f|o|g]."""
    perm = np.zeros(4 * H, dtype=np.int64)
    for j in range(NG):
        base = j * 1024
        hid = np.arange(256) + j * 256
        perm[base + 0:base + 256] = 0 * H + hid      # i
        perm[base + 256:base + 512] = 1 * H + hid    # f
        perm[base + 512:base + 768] = 3 * H + hid    # o
        perm[base + 768:base + 1024] = 2 * H + hid   # g
    return perm


def prep_inputs(hs_pad, ys_in_pad, embed, W_ih0, W_hh0, b_ih0, b_hh0,
                W_ih1, W_hh1, b_ih1, b_hh1, W_enc, b_enc, W_dec, W_out, b_out,
                U=64, n_cores=8):
    f16 = ml_dtypes.float16 if not hasattr(np, "float16") else np.float16
    perm = gate_perm()

    def wiht(W, KD, KC):  # (4H, KD) -> (128, KC, 4096) fp16, permuted gates
        Wp = W[perm]                      # (4096, KD)
        return np.ascontiguousarray(
            Wp.T.reshape(KC, 128, 4096).transpose(1, 0, 2)).astype(np.float16)

    def whht(W):  # (4H, H) -> (128, HK, NG, 1024) fp16
        Wp = W[perm]                      # (4096, 1024) rows=permuted gates
        # [p, kc, j, n] = Wp[j*1024+n, kc*128+p]
        a = Wp.T.reshape(HK, 128, NG, 1024).transpose(1, 0, 2, 3)
        return np.ascontiguousarray(a).astype(np.float16)

    ins = {}
    ins["embed"] = np.asarray(embed, np.float32)
    ys = np.asarray(ys_in_pad).astype(np.int32)   # (B, U)
    NCH = B * U // 128
    yy = np.zeros((128, NCH), np.int32)
    for ch in range(NCH):
        p = np.arange(128)
        yy[:, ch] = ys[p % 8, ch * 16 + p // 8]
    ins["yidx"] = yy
    ins["wih0t"] = wiht(W_ih0, E, EK)
    w1 = wiht(W_ih1, H, HK)  # (128, HK, 4096)
    ins["wih1t"] = np.ascontiguousarray(
        w1.reshape(128, HK, 16, 256).transpose(2, 0, 1, 3))
    ins["whh0t"] = whht(W_hh0)
    ins["whh1t"] = whht(W_hh1)
    ins["inj8"] = np.eye(8, dtype=np.float16)
    ins["eye128"] = np.eye(128, dtype=np.float16)
    # [p, ec, jc, m] = W[jc*128+m, ec*128+p]
    def wjt(W, KC):
        a = W.T.reshape(KC, 128, JC, 128).transpose(1, 0, 2, 3)
        return np.ascontiguousarray(a).astype(np.float16)
    ins["wenct"] = wjt(W_enc, HK)
    ins["wdect"] = wjt(W_dec, HK)
    # [p, jc, od] = W_out[od, jc*128+p]
    ins["woutt"] = np.ascontiguousarray(
        W_out.T.reshape(JC, 128, OD).transpose(1, 0, 2)).astype(np.float16)
    ins["benc"] = np.ascontiguousarray(
        b_enc.reshape(JC, 128).T).astype(np.float32)
    ins["boutrep"] = np.tile(np.asarray(b_out, np.float32)[None, :], (128, 1))
    ins["bihh0"] = np.tile(((b_ih0 + b_hh0)[perm]).astype(np.float16)[None, :],
                           (128, 1))
    ins["bihh1"] = np.tile(((b_ih1 + b_hh1)[perm]).astype(np.float16)[None, :],
                           (128, 1))

    maps = []
    for c in range(n_cores):
        m = dict(ins)
        # [p, ec, r] = hs[b, TSH*c + tl, ec*128+p], r = b*TSH+tl
        sl = np.asarray(hs_pad[:, TSH * c:TSH * (c + 1), :], np.float32)
        a = sl.reshape(B * TSH, HK, 128).transpose(2, 1, 0)
        m["hst16"] = np.ascontiguousarray(a).astype(np.float16)
        maps.append(m)
    return maps


def gather_output(results):
    outs = [np.asarray(r["out"], np.float32).reshape(B, TSH, -1, OD)
            for r in results]
    return np.concatenate(outs, axis=1)


# ---------------- entry point ----------------
import sys as _sys
import types as _types

# Recreate the missing antenv.axon_hooks so trace=True works under axon
# (used only when BASS_TRACE=1 is set by a profiling harness).
if "antenv.axon_hooks" not in _sys.modules:
    _m = _types.ModuleType("antenv.axon_hooks")

    def _get_hook():
        try:
            from trn_agent_boot.trn_boot import _ntff_profile_via_ctypes
            return _ntff_profile_via_ctypes("/opt/axon/libaxon_pjrt.so")
        except Exception:
            return None
    _m.get_axon_ntff_profile_hook = _get_hook
    _sys.modules["antenv.axon_hooks"] = _m

_NC = None
last_results = None


def kernel(**inputs):
    """Full-input RNN-T decoder: returns (B, T, U, ODIM) float32."""
    global _NC, last_results
    from concourse.bass_utils import run_bass_kernel_spmd
    U = int(np.asarray(inputs["ys_in_pad"]).shape[1])
    wb = any(float(np.abs(np.asarray(inputs[k])).max()) != 0.0
             for k in ("b_ih0", "b_hh0", "b_ih1", "b_hh1"))
    wob = float(np.abs(np.asarray(inputs["b_out"])).max()) != 0.0
    if _NC is None:
        _NC = build_program(U=U, n_cores=8, with_biases=wb, with_out_bias=wob)
    maps = prep_inputs(**inputs, U=U)
    res = run_bass_kernel_spmd(_NC, maps, core_ids=list(range(8)))
    last_results = res
    return gather_output(res.results)



# revision 33
# speedup vs baseline: 1.2027x; 1.0141x over previous
"""RNN-T decoder kernel for TRN2 (8 cores, T-sharded joint, replicated LSTM).

Layout notes
------------
B=8, T=128, U=64, E=512, H=1024 (8 k-chunks), J=640 (5 j-chunks), OD=1024.
Each core handles T-slice [16c, 16c+16) of the joint; the 2-layer LSTM over U
is computed identically (replicated, all 8 batches) on every core.

Gate permutation: hidden dim is split in 4 quarters (col-tile groups). Group
j's 1024 gate columns are [i_j | f_j | o_j | g_j] (256 each), where x_j acts
on hidden units [256j, 256j+256). Weights/bias/X tensors are host-permuted
to this order.

Gates PSUM tile (128, 1024): group j occupies partitions [32j, 32j+8)
(batch-major), accumulated by 4-way column-packed matmuls (tile_position).

Schedule: software-pipelined wavefronts with layer-1 lagging LAG=18 steps.
Per wavefront the PE runs [T(l0,u-1), R(l0,u), T(l1,v-1), R(l1,v)] back to
back; each layer's activation chain (ACT+DVE) is emitted after all PE work
so neither layer's chain gates the other's stream. X0 is produced in
u-blocks ahead of consumption (block 0 up front, the rest one weight-chunk
per early wavefront); X1 blocks are split in 4-chunk sub-blocks spread over
2 wavefronts with sync/scalar dual-queue fp8 weight loads. h0 history is
kept in both f16 (recurrence) and fp8 (X1 matmuls). The joint runs after
the LSTM with fused broadcast-add/tanh over all jc and f16 weights/output.
"""
import numpy as np
import ml_dtypes

import concourse.bass as bass
import concourse.bacc as bacc
import concourse.mybir as mybir
import concourse.tile as tile

dt = mybir.dt
AF = mybir.ActivationFunctionType

B, T, E, H, J, OD = 8, 128, 512, 1024, 640, 1024
HK = H // 128   # 8 h-chunks
JC = J // 128   # 5 j-chunks
EK = E // 128   # 4 e-chunks
TSH = T // 8    # 16 t per core
NG = 4          # col-tile groups


def bcast_mid(ap, count):
    """(128, N) AP -> (128, count, N) with a 0-step middle dim."""
    return bass.AP(ap.tensor, ap.offset, [ap.ap[0], [0, count], ap.ap[1]])


def build_program(U=64, n_cores=8, with_biases=False,
                  with_out_bias=False):
    nc = bacc.Bacc("TRN2", target_bir_lowering=False, debug=False,
                   num_devices=n_cores)
    f16, f32, f32r, i32 = dt.float16, dt.float32, dt.float32r, dt.int32
    UG = U // 16  # u-blocks of 16
    assert U % 16 == 0

    # ---------------- external inputs ----------------
    embed_d = nc.dram_tensor("embed", [OD, E], f32, kind="ExternalInput")
    yidx_d = nc.dram_tensor("yidx", [128, B * U // 128], i32, kind="ExternalInput")
    wih0_d = nc.dram_tensor("wih0t", [128, EK, 4096], f16, kind="ExternalInput")
    wih1_d = nc.dram_tensor("wih1t", [16, 128, HK, 256], f16, kind="ExternalInput")
    whh0_d = nc.dram_tensor("whh0t", [128, HK, NG, 1024], f16, kind="ExternalInput")
    whh1_d = nc.dram_tensor("whh1t", [128, HK, NG, 1024], f16, kind="ExternalInput")
    inj_d = nc.dram_tensor("inj8", [8, 8], f16, kind="ExternalInput")
    eye128_d = nc.dram_tensor("eye128", [128, 128], f16, kind="ExternalInput")
    wenc_d = nc.dram_tensor("wenct", [128, HK, JC, 128], f16, kind="ExternalInput")
    wdec_d = nc.dram_tensor("wdect", [128, HK, JC, 128], f16, kind="ExternalInput")
    wout_d = nc.dram_tensor("woutt", [128, JC, OD], f16, kind="ExternalInput")
    benc_d = nc.dram_tensor("benc", [128, JC], f32, kind="ExternalInput")
    bout_d = nc.dram_tensor("boutrep", [128, OD], f32, kind="ExternalInput")
    hst_d = nc.dram_tensor("hst16", [128, HK, B * TSH], f16, kind="ExternalInput")
    # per-layer (b_ih + b_hh), gate-permuted, replicated over partitions
    bi0_d = nc.dram_tensor("bihh0", [128, 4096], f16, kind="ExternalInput")
    bi1_d = nc.dram_tensor("bihh1", [128, 4096], f16, kind="ExternalInput")

    out_d = nc.dram_tensor("out", [B * TSH * U, OD], f16, kind="ExternalOutput")

    # ---------------- internal dram ----------------
    x0_d = nc.dram_tensor("X0d", [U, B, 4096], f16)
    x1_d = nc.dram_tensor("X1d", [U, B, 4096], f16)

    LAG = 20  # layer-1 runs LAG wavefronts behind layer-0

    with tile.TileContext(nc) as tc:
        with tc.tile_pool(name="const", bufs=1) as pc:
            # constants (small, urgent loads first on the sync queue)
            eye128_sb = pc.tile([128, 128], f16, tag="eye128")
            nc.sync.dma_start(eye128_sb[:], eye128_d.ap())
            yidx_sb = pc.tile([128, B * U // 128], i32, tag="yidx")
            nc.sync.dma_start(yidx_sb[:], yidx_d.ap())
            inj_sb = pc.tile([8, 8], f16, tag="inj")
            nc.sync.dma_start(inj_sb[:], inj_d.ap())
            hst_sb = pc.tile([128, HK, B * TSH], f16, tag="hst")
            nc.scalar.dma_start(hst_sb[:], hst_d.ap())
            if with_biases:
                bi0_sb = pc.tile([128, 4096], f16, tag="bi0")
                nc.scalar.dma_start(bi0_sb[:], bi0_d.ap())
                bi1_sb = pc.tile([128, 4096], f16, tag="bi1")
                nc.scalar.dma_start(bi1_sb[:], bi1_d.ap())
            # h_dec transposed history, both layers (fp16)
            hdec = [pc.tile([128, HK, U, B], f16, tag=f"hdec{l}",
                            name=f"hdec{l}") for l in range(2)]

            pw_ctx = tc.tile_pool(name="whh", bufs=1)
            pw = pw_ctx.__enter__()
            # recurrent weights (resident for whole LSTM); per-kc chunked
            # loads on the vector queue so early chunks arrive early and
            # don't block the sync queue.
            whh_sb = [pw.tile([128, HK, NG, 1024], f16, tag=f"whh{l}",
                               name=f"whh{l}") for l in range(2)]
            # whh0 loads are emitted after x0_block(0) (same scalar queue)
            # so the startup-critical X0 weight chunks go first; whh1 loads
            # are deferred into the wavefront loop (needed from wavefront
            # LAG on).

            # ---------------- main pools (LSTM + pre phases) --------------
            with (
                tc.tile_pool(name="lstmS", bufs=1) as lS,
                tc.tile_pool(name="lstmPS", bufs=1, space="PSUM") as lP,
            ):
                # ---------------- P1: embedding gather + eys^T ------------
                sc = nc.named_scope("gather"); sc.__enter__()
                NCH = B * U // 128  # row chunks of 128
                eyst = lS.tile([128, EK, B * U], f16, tag="eyst")
                for ch in range(NCH):
                    g32 = lS.tile([128, E], f32, tag="g32", bufs=1)
                    nc.gpsimd.indirect_dma_start(
                        out=g32[:], out_offset=None, in_=embed_d.ap(),
                        in_offset=bass.IndirectOffsetOnAxis(
                            ap=yidx_sb[:, ch:ch + 1], axis=0))
                    g16 = lS.tile([128, E], f16, tag="g16", bufs=1)
                    nc.vector.tensor_copy(g16[:], g32[:])
                    for ec in range(EK):
                        tp = lP.tile([128, 128], f16, tag="tp128", bufs=2)
                        nc.tensor.transpose(
                            tp[:], g16[:, ec * 128:(ec + 1) * 128], eye128_sb[:])
                        nc.vector.tensor_copy(
                            eyst[:, ec, ch * 128:(ch + 1) * 128], tp[:])
                sc.__exit__(None, None, None)

                gate_ps = [lP.tile([128, 1024], f32, tag=f"gates{l}",
                                   name=f"gates{l}") for l in range(2)]
                nc.vector.memset(gate_ps[0][:], 0.0)
                nc.vector.memset(gate_ps[1][:], 0.0)
                czero = [lS.tile([128, 256], f32, tag=f"c{l}", name=f"cz{l}",
                                  bufs=2) for l in range(2)]
                nc.gpsimd.memset(czero[0][:], 0.0)
                nc.gpsimd.memset(czero[1][:], 0.0)
                cprev = [czero[0], czero[1]]
                xsrc = [x0_d, x1_d]

                # ---------------- P2: X0 u-blocks -------------------------
                def x0_block(g):
                    # X0[u-block g] = eys-block @ W_ih0^T, streamed weights
                    for nc_ in range(8):
                        w0c = lS.tile([128, EK, 512], f16, tag="w0c", bufs=2)
                        eng = nc.sync if nc_ % 2 == 0 else nc.scalar
                        eng.dma_start(
                            w0c[:],
                            wih0_d.ap()[:, :, nc_ * 512:(nc_ + 1) * 512])
                        ps = lP.tile([128, 512], f32, tag="xps", bufs=2)
                        for ec in range(EK):
                            nc.tensor.matmul(
                                ps[:],
                                eyst[:, ec, g * 128:(g + 1) * 128],
                                w0c[:, ec, :],
                                start=(ec == 0), stop=(ec == EK - 1))
                        x0c = lS.tile([128, 512], f16, tag="x0c", bufs=2)
                        if with_biases:
                            nc.vector.tensor_add(
                                x0c[:], ps[:],
                                bi0_sb[:, nc_ * 512:(nc_ + 1) * 512])
                        else:
                            nc.vector.tensor_copy(x0c[:], ps[:])
                        nc.sync.dma_start(
                            x0_d.ap()[g * 16:(g + 1) * 16, :,
                                      nc_ * 512:(nc_ + 1) * 512],
                            x0c[:])

                def x0_rest(nc_):
                    # one W_ih0 chunk, X0 for u-blocks 1..3 (loads W once)
                    w0c = lS.tile([128, EK, 512], f16, tag="w0c", bufs=2)
                    eng = nc.sync if nc_ % 2 == 0 else nc.scalar
                    eng.dma_start(
                        w0c[:], wih0_d.ap()[:, :, nc_ * 512:(nc_ + 1) * 512])
                    for g in range(1, UG):
                        ps = lP.tile([128, 512], f32, tag="xps", bufs=2)
                        for ec in range(EK):
                            nc.tensor.matmul(
                                ps[:],
                                eyst[:, ec, g * 128:(g + 1) * 128],
                                w0c[:, ec, :],
                                start=(ec == 0), stop=(ec == EK - 1))
                        x0c = lS.tile([128, 512], f16, tag="x0c", bufs=2)
                        if with_biases:
                            nc.vector.tensor_add(
                                x0c[:], ps[:],
                                bi0_sb[:, nc_ * 512:(nc_ + 1) * 512])
                        else:
                            nc.vector.tensor_copy(x0c[:], ps[:])
                        nc.sync.dma_start(
                            x0_d.ap()[g * 16:(g + 1) * 16, :,
                                      nc_ * 512:(nc_ + 1) * 512],
                            x0c[:])

                # ---------------- P3: LSTM pieces -------------------------
                def lstm_rec(l, u):
                    # inject + recurrent matmuls into gates psum [PE].
                    # inject first: it has no h-dependency, so it can run
                    # during the previous step's activation chain.
                    pg = gate_ps[l]
                    xf = lS.tile([8, 4096], f16, tag="xf", bufs=2)
                    nc.gpsimd.dma_start(xf[:], xsrc[l].ap()[u])
                    for hf in range(2):
                        sl = slice(hf * 512, (hf + 1) * 512)
                        for j in range(NG):
                            nc.tensor.matmul(
                                pg[32 * j:32 * j + 8, sl], inj_sb[:],
                                xf[:, j * 1024 + hf * 512:
                                   j * 1024 + (hf + 1) * 512],
                                tile_position=(0, 32 * j),
                                start=True, stop=(u == 0))
                    if u > 0:
                        for kc in range(HK):
                            for hf in range(2):
                                sl = slice(hf * 512, (hf + 1) * 512)
                                for j in range(NG):
                                    nc.tensor.matmul(
                                        pg[32 * j:32 * j + 8, sl],
                                        hdec[l][:, kc, u - 1, :],
                                        whh_sb[l][:, kc, j,
                                                  hf * 512:(hf + 1) * 512],
                                        tile_position=(0, 32 * j),
                                        start=False, stop=(kc == HK - 1))

                hbuf = [None, None]  # last h tile per layer

                def lstm_chain(l, u, solo=False):
                    # gates -> sigmoid/tanh -> c,h  [ACT + DVE only]
                    # sigmoid split i,f | o so the c-path starts ~0.4us
                    # earlier.
                    pg = gate_ps[l]
                    sig = lS.tile([128, 768], f16, tag=f"sig{l}")
                    nc.scalar.activation(sig[:, 0:512], pg[:, 0:512],
                                         AF.Sigmoid)
                    tg = lS.tile([128, 256], f16, tag=f"tg{l}")
                    nc.scalar.activation(tg[:], pg[:, 768:1024], AF.Tanh)
                    nc.scalar.activation(sig[:, 512:768], pg[:, 512:768],
                                         AF.Sigmoid)
                    cnew = lS.tile([128, 256], f32, tag=f"c{l}", bufs=2)
                    nc.vector.tensor_mul(cnew[:], sig[:, 256:512], cprev[l][:])
                    t1 = lS.tile([128, 256], f32, tag=f"t1{l}")
                    nc.vector.tensor_mul(t1[:], sig[:, 0:256], tg[:])
                    nc.vector.tensor_add(cnew[:], cnew[:], t1[:])
                    cprev[l] = cnew
                    tc_ = lS.tile([128, 256], f16, tag=f"tc{l}")
                    nc.scalar.activation(tc_[:], cnew[:], AF.Tanh)
                    h = lS.tile([128, 256], f16, tag=f"h{l}", bufs=2)
                    nc.vector.tensor_mul(h[:], sig[:, 512:768], tc_[:])
                    hbuf[l] = h

                def lstm_transp(l, u):
                    # h -> hdec[l][:, :, u, :]  [PE transpose + DVE copy]
                    h = hbuf[l]
                    for cb in range(2):
                        tp = lP.tile([128, 128], f16, tag="tp128", bufs=2)
                        nc.tensor.transpose(
                            tp[:], h[:, cb * 128:(cb + 1) * 128],
                            eye128_sb[:])
                        hd = hdec[l][:, 0, u, :]  # (128, B) at kc=0
                        dst = bass.AP(hd.tensor, hd.offset + cb * U * B,
                                      [hd.ap[0], [2 * U * B, NG], [1, B]])
                        src_ap = bass.AP(tp[:].tensor, tp[:].offset,
                                         [tp[:].ap[0], [32, NG], [1, B]])
                        nc.vector.tensor_copy(dst, src_ap)

                def x1_subblock(kb, sb):
                    # 4 of 16 weight chunks of X1 u-block kb; loads alternate
                    # sync/scalar DMA queues to double effective bandwidth
                    hd0 = hdec[0]
                    for nc2 in range(4 * sb, 4 * sb + 4):
                        w1c = lS.tile([128, HK, 256], f16, tag="w1c", bufs=2)
                        eng = nc.sync if nc2 % 2 == 0 else nc.scalar
                        eng.dma_start(w1c[:], wih1_d.ap()[nc2])
                        ps = lP.tile([128, 512], f32, tag="xps", bufs=2)
                        for kc in range(HK):
                            nc.tensor.matmul(
                                ps[:, 0:256],
                                hd0[:, kc, kb * 16:(kb + 1) * 16, :],
                                w1c[:, kc, :],
                                start=(kc == 0), stop=(kc == HK - 1))
                        x1c = lS.tile([128, 256], f16, tag="x1c", bufs=2)
                        if with_biases:
                            nc.vector.tensor_add(
                                x1c[:], ps[:, 0:256],
                                bi1_sb[:, nc2 * 256:(nc2 + 1) * 256])
                        else:
                            nc.vector.tensor_copy(x1c[:], ps[:, 0:256])
                        nc.sync.dma_start(
                            x1_d.ap()[kb * 16:(kb + 1) * 16, :,
                                      nc2 * 256:(nc2 + 1) * 256],
                            x1c[:])

                # early ze: encoder projection needs only hst + wenc;
                # computed during the l1-solo tail to fill PE idle there
                benc_sb = lS.tile([128, JC], f32, tag="bencs")
                ze_sb = lS.tile([128, JC, B * TSH], f16, tag="ze")

                def ze_compute():
                    nc.sync.dma_start(benc_sb[:], benc_d.ap())
                    for jc in range(JC):
                        wencc = lS.tile([128, HK, 128], f16, tag="wencc",
                                        bufs=1)
                        eng = nc.sync if jc % 2 == 0 else nc.scalar
                        eng.dma_start(wencc[:], wenc_d.ap()[:, :, jc, :])
                        zp = lP.tile([128, 512], f32, tag="xps", bufs=2)
                        for ec in range(HK):
                            nc.tensor.matmul(zp[:, 0:128],
                                             wencc[:, ec, :],
                                             hst_sb[:, ec, :],
                                             start=(ec == 0),
                                             stop=(ec == HK - 1))
                        nc.scalar.activation(ze_sb[:, jc, :], zp[:, 0:128],
                                             AF.Identity,
                                             bias=benc_sb[:, jc:jc + 1])

                # ---------------- wavefront schedule ----------------------
                # per wavefront w:
                #   PE:  T(l0,w-1) R(l0,w) T(l1,w-1-LAG) R(l1,w-LAG) [x1blk]
                #   chains emitted after all PE work so neither layer's
                #   ACT/DVE ops gate the other layer's PE stream.
                with nc.named_scope("x0b0"):
                    x0_block(0)
                for kc in range(HK):
                    eng = nc.sync if kc % 2 == 0 else nc.scalar
                    eng.dma_start(whh_sb[0][:, kc, :, :],
                                  whh0_d.ap()[:, kc, :, :])
                for w in range(U + LAG + 1):
                    u0, u1 = w, w - LAG
                    with nc.named_scope(f"w{w:02d}"):
                        if 1 <= u0 <= U:
                            lstm_transp(0, u0 - 1)
                        if u0 < U:
                            lstm_rec(0, u0)
                        if 1 <= u1 <= U:
                            lstm_transp(1, u1 - 1)
                        if 0 <= u1 < U:
                            lstm_rec(1, u1)
                        dual = (u0 < U) and (0 <= u1 < U)
                        if u0 < U:
                            lstm_chain(0, u0, solo=not dual)
                        if 0 <= u1 < U:
                            lstm_chain(1, u1, solo=not dual)
                    if 1 <= u0 <= 8:
                        with nc.named_scope(f"x0r{u0 - 1}"):
                            x0_rest(u0 - 1)
                    if u0 == 10:
                        for kc in range(HK):
                            nc.scalar.dma_start(whh_sb[1][:, kc, :, :],
                                                whh1_d.ap()[:, kc, :, :])
                    if u0 == U + 2:
                        with nc.named_scope("ze"):
                            ze_compute()
                    if 16 <= u0 < 68 and (u0 - 16) % 16 < 4:
                        kb, sb = (u0 - 16) // 16, (u0 - 16) % 16
                        with nc.named_scope(f"x1b{kb}_{sb}"):
                            x1_subblock(kb, sb)

            pw_ctx.__exit__(None, None, None)

            # ---------------- P4/P5: joint ----------------
            sc_j = nc.named_scope("joint"); sc_j.__enter__()
            with (
                tc.tile_pool(name="jS", bufs=1) as jS,
                tc.tile_pool(name="jPS", bufs=1, space="PSUM") as jP,
            ):
                wdec_sb = jS.tile([128, HK, JC, 128], f16, tag="wdec")
                nc.sync.dma_start(wdec_sb[:], wdec_d.ap())
                wout_sb = jS.tile([128, JC, OD], f16, tag="woutr")
                nc.sync.dma_start(wout_sb[:], wout_d.ap())
                if with_out_bias:
                    bout_sb = jS.tile([128, OD], f32, tag="bouts")
                    nc.sync.dma_start(bout_sb[:], bout_d.ap())

                # ze (J, b*tl) and zd (J, b, u)
                sc_z = nc.named_scope("zedzd"); sc_z.__enter__()
                zd_sb = jS.tile([128, JC, U, B], f16, tag="zd")
                for jc in range(JC):
                    zp = jP.tile([128, U * B], f32, tag="zdps", bufs=2)
                    for kc in range(HK):
                        nc.tensor.matmul(
                            zp[:], wdec_sb[:, kc, jc, :],
                            hdec[1][:, kc, :, :].rearrange("p u b -> p (u b)"),
                            start=(kc == 0), stop=(kc == HK - 1))
                    nc.vector.tensor_copy(
                        zd_sb[:, jc, :, :].rearrange("p u b -> p (u b)"), zp[:])

                sc_z.__exit__(None, None, None)
                sc_b = nc.named_scope("jblk"); sc_b.__enter__()
                # joint blocks: 128 rows = 2 (b,tl) pairs x U
                n_pairs = B * TSH
                rows_per_pair = U
                ppb = 128 // rows_per_pair  # pairs per block
                BTSH = B * TSH
                for blk in range(n_pairs // ppb):
                    pr0 = blk * ppb
                    b = pr0 // TSH
                    zjt = jS.tile([128, JC, 128], f16, tag="zjt", bufs=2)
                    zj = jS.tile([128, JC, 128], f16, tag="zj", bufs=2)
                    # fused over all jc: out[p, jc, a, u] = ze[p, jc, pr0+a]
                    #                                     + zd[p, jc, u, b]
                    zjt_ap = zjt[:, :, :].rearrange(
                        "p jc (a u) -> p jc a u", a=ppb)
                    zea = ze_sb[:, 0, 0]
                    ze_bc = bass.AP(zea.tensor, zea.offset + pr0,
                                    [zea.ap[0], [BTSH, JC], [1, ppb], [0, U]])
                    zda = zd_sb[:, 0, 0, 0]
                    zd_bc = bass.AP(zda.tensor, zda.offset + b,
                                    [zda.ap[0], [U * B, JC], [0, ppb], [B, U]])
                    nc.vector.tensor_tensor(
                        zjt_ap, ze_bc, zd_bc, op=mybir.AluOpType.add)
                    nc.scalar.activation(
                        zj[:, :, :].rearrange("p jc m -> p (jc m)"),
                        zjt[:, :, :].rearrange("p jc m -> p (jc m)"), AF.Tanh)
                    ops_ = jP.tile([128, OD], f32, tag="outps", bufs=2)
                    for n2 in range(2):
                        for jc in range(JC):
                            nc.tensor.matmul(
                                ops_[:, n2 * 512:(n2 + 1) * 512],
                                zj[:, jc, :],
                                wout_sb[:, jc, n2 * 512:(n2 + 1) * 512],
                                start=(jc == 0), stop=(jc == JC - 1))
                    osb = jS.tile([128, OD], f16, tag="osb", bufs=3)
                    if with_out_bias:
                        nc.vector.tensor_add(osb[:], ops_[:], bout_sb[:])
                    else:
                        # split the psum->sbuf evacuation across DVE + ACT
                        # so neither engine bottlenecks the block pipeline
                        nc.vector.tensor_copy(osb[:, 0:512], ops_[:, 0:512])
                        nc.scalar.activation(osb[:, 512:1024],
                                             ops_[:, 512:1024], AF.Identity)
                    eng = nc.sync if blk % 2 == 0 else nc.scalar
                    eng.dma_start(
                        out_d.ap()[blk * 128:(blk + 1) * 128, :], osb[:])
                sc_b.__exit__(None, None, None)
            sc_j.__exit__(None, None, None)

    nc.compile()
    return nc


# ---------------- host-side prep ----------------

def gate_perm():
    """perm[j*1024 + s] -> row index in torch (i,f,g,o) 4H gate layout,
    with group-local order [i|f|o|g]."""
    perm = np.zeros(4 * H, dtype=np.int64)
    for j in range(NG):
        base = j * 1024
        hid = np.arange(256) + j * 256
        perm[base + 0:base + 256] = 0 * H + hid      # i
        perm[base + 256:base + 512] = 1 * H + hid    # f
        perm[base + 512:base + 768] = 3 * H + hid    # o
        perm[base + 768:base + 1024] = 2 * H + hid   # g
    return perm


def prep_inputs(hs_pad, ys_in_pad, embed, W_ih0, W_hh0, b_ih0, b_hh0,
                W_ih1, W_hh1, b_ih1, b_hh1, W_enc, b_enc, W_dec, W_out, b_out,
                U=64, n_cores=8):
    f16 = ml_dtypes.float16 if not hasattr(np, "float16") else np.float16
    perm = gate_perm()

    def wiht(W, KD, KC):  # (4H, KD) -> (128, KC, 4096) fp16, permuted gates
        Wp = W[perm]                      # (4096, KD)
        return np.ascontiguousarray(
            Wp.T.reshape(KC, 128, 4096).transpose(1, 0, 2)).astype(np.float16)

    def whht(W):  # (4H, H) -> (128, HK, NG, 1024) fp16
        Wp = W[perm]                      # (4096, 1024) rows=permuted gates
        # [p, kc, j, n] = Wp[j*1024+n, kc*128+p]
        a = Wp.T.reshape(HK, 128, NG, 1024).transpose(1, 0, 2, 3)
        return np.ascontiguousarray(a).astype(np.float16)

    ins = {}
    ins["embed"] = np.asarray(embed, np.float32)
    ys = np.asarray(ys_in_pad).astype(np.int32)   # (B, U)
    NCH = B * U // 128
    yy = np.zeros((128, NCH), np.int32)
    for ch in range(NCH):
        p = np.arange(128)
        yy[:, ch] = ys[p % 8, ch * 16 + p // 8]
    ins["yidx"] = yy
    ins["wih0t"] = wiht(W_ih0, E, EK)
    w1 = wiht(W_ih1, H, HK)  # (128, HK, 4096)
    ins["wih1t"] = np.ascontiguousarray(
        w1.reshape(128, HK, 16, 256).transpose(2, 0, 1, 3))
    ins["whh0t"] = whht(W_hh0)
    ins["whh1t"] = whht(W_hh1)
    ins["inj8"] = np.eye(8, dtype=np.float16)
    ins["eye128"] = np.eye(128, dtype=np.float16)
    # [p, ec, jc, m] = W[jc*128+m, ec*128+p]
    def wjt(W, KC):
        a = W.T.reshape(KC, 128, JC, 128).transpose(1, 0, 2, 3)
        return np.ascontiguousarray(a).astype(np.float16)
    ins["wenct"] = wjt(W_enc, HK)
    ins["wdect"] = wjt(W_dec, HK)
    # [p, jc, od] = W_out[od, jc*128+p]
    ins["woutt"] = np.ascontiguousarray(
        W_out.T.reshape(JC, 128, OD).transpose(1, 0, 2)).astype(np.float16)
    ins["benc"] = np.ascontiguousarray(
        b_enc.reshape(JC, 128).T).astype(np.float32)
    ins["boutrep"] = np.tile(np.asarray(b_out, np.float32)[None, :], (128, 1))
    ins["bihh0"] = np.tile(((b_ih0 + b_hh0)[perm]).astype(np.float16)[None, :],
                           (128, 1))
    ins["bihh1"] = np.tile(((b_ih1 + b_hh1)[perm]).astype(np.float16)[None, :],
                           (128, 1))

    maps = []
    for c in range(n_cores):
        m = dict(ins)
        # [p, ec, r] = hs[b, TSH*c + tl, ec*128+p], r = b*TSH+tl
        sl = np.asarray(hs_pad[:, TSH * c:TSH * (c + 1), :], np.float32)
        a = sl.reshape(B * TSH, HK, 128).transpose(2, 1, 0)
        m["hst16"] = np.ascontiguousarray(a).astype(np.float16)
        maps.append(m)
    return maps


def gather_output(results):
    outs = [np.asarray(r["out"], np.float32).reshape(B, TSH, -1, OD)
            for r in results]
    return np.concatenate(outs, axis=1)


# ---------------- entry point ----------------
import sys as _sys
import types as _types

# Recreate the missing antenv.axon_hooks so trace=True works under axon
# (used only when BASS_TRACE=1 is set by a profiling harness).
if "antenv.axon_hooks" not in _sys.modules:
    _m = _types.ModuleType("antenv.axon_hooks")

    def _get_hook():
        try:
            from trn_agent_boot.trn_boot import _ntff_profile_via_ctypes
            return _ntff_profile_via_ctypes("/opt/axon/libaxon_pjrt.so")
        except Exception:
            return None
    _m.get_axon_ntff_profile_hook = _get_hook
    _sys.modules["antenv.axon_hooks"] = _m

_NC = None
last_results = None


def kernel(**inputs):
    """Full-input RNN-T decoder: returns (B, T, U, ODIM) float32."""
    global _NC, last_results
    from concourse.bass_utils import run_bass_kernel_spmd
    U = int(np.asarray(inputs["ys_in_pad"]).shape[1])
    wb = any(float(np.abs(np.asarray(inputs[k])).max()) != 0.0
             for k in ("b_ih0", "b_hh0", "b_ih1", "b_hh1"))
    wob = float(np.abs(np.asarray(inputs["b_out"])).max()) != 0.0
    if _NC is None:
        _NC = build_program(U=U, n_cores=8, with_biases=wb, with_out_bias=wob)
    maps = prep_inputs(**inputs, U=U)
    res = run_bass_kernel_spmd(_NC, maps, core_ids=list(range(8)))
    last_results = res
    return gather_output(res.results)



# revision 34
# speedup vs baseline: 1.2148x; 1.0100x over previous
"""RNN-T decoder kernel for TRN2 (8 cores, T-sharded joint, replicated LSTM).

Layout notes
------------
B=8, T=128, U=64, E=512, H=1024 (8 k-chunks), J=640 (5 j-chunks), OD=1024.
Each core handles T-slice [16c, 16c+16) of the joint; the 2-layer LSTM over U
is computed identically (replicated, all 8 batches) on every core.

Gate permutation: hidden dim is split in 4 quarters (col-tile groups). Group
j's 1024 gate columns are [i_j | f_j | o_j | g_j] (256 each), where x_j acts
on hidden units [256j, 256j+256). Weights/bias/X tensors are host-permuted
to this order.

Gates PSUM tile (128, 1024): group j occupies partitions [32j, 32j+8)
(batch-major), accumulated by 4-way column-packed matmuls (tile_position).

Schedule: software-pipelined wavefronts with layer-1 lagging LAG=18 steps.
Per wavefront the PE runs [T(l0,u-1), R(l0,u), T(l1,v-1), R(l1,v)] back to
back; each layer's activation chain (ACT+DVE) is emitted after all PE work
so neither layer's chain gates the other's stream. X0 is produced in
u-blocks ahead of consumption (block 0 up front, the rest one weight-chunk
per early wavefront); X1 blocks are split in 4-chunk sub-blocks spread over
2 wavefronts with sync/scalar dual-queue fp8 weight loads. h0 history is
kept in both f16 (recurrence) and fp8 (X1 matmuls). The joint runs after
the LSTM with fused broadcast-add/tanh over all jc and f16 weights/output.
"""
import numpy as np
import ml_dtypes

import concourse.bass as bass
import concourse.bacc as bacc
import concourse.mybir as mybir
import concourse.tile as tile

dt = mybir.dt
AF = mybir.ActivationFunctionType

B, T, E, H, J, OD = 8, 128, 512, 1024, 640, 1024
HK = H // 128   # 8 h-chunks
JC = J // 128   # 5 j-chunks
EK = E // 128   # 4 e-chunks
TSH = T // 8    # 16 t per core
NG = 4          # col-tile groups


def bcast_mid(ap, count):
    """(128, N) AP -> (128, count, N) with a 0-step middle dim."""
    return bass.AP(ap.tensor, ap.offset, [ap.ap[0], [0, count], ap.ap[1]])


def build_program(U=64, n_cores=8, with_biases=False,
                  with_out_bias=False):
    nc = bacc.Bacc("TRN2", target_bir_lowering=False, debug=False,
                   num_devices=n_cores)
    f16, f32, f32r, i32 = dt.float16, dt.float32, dt.float32r, dt.int32
    UG = U // 16  # u-blocks of 16
    assert U % 16 == 0

    # ---------------- external inputs ----------------
    embed_d = nc.dram_tensor("embed", [OD, E], f32, kind="ExternalInput")
    yidx_d = nc.dram_tensor("yidx", [128, B * U // 128], i32, kind="ExternalInput")
    wih0_d = nc.dram_tensor("wih0t", [128, EK, 4096], f16, kind="ExternalInput")
    wih1_d = nc.dram_tensor("wih1t", [16, 128, HK, 256], f16, kind="ExternalInput")
    whh0_d = nc.dram_tensor("whh0t", [128, HK, NG, 1024], f16, kind="ExternalInput")
    whh1_d = nc.dram_tensor("whh1t", [128, HK, NG, 1024], f16, kind="ExternalInput")
    inj_d = nc.dram_tensor("inj8", [8, 8], f8, kind="ExternalInput")
    eye128_d = nc.dram_tensor("eye128", [128, 128], f16, kind="ExternalInput")
    wenc_d = nc.dram_tensor("wenct", [128, HK, JC, 128], f16, kind="ExternalInput")
    wdec_d = nc.dram_tensor("wdect", [128, HK, JC, 128], f16, kind="ExternalInput")
    wout_d = nc.dram_tensor("woutt", [128, JC, OD], f16, kind="ExternalInput")
    benc_d = nc.dram_tensor("benc", [128, JC], f32, kind="ExternalInput")
    bout_d = nc.dram_tensor("boutrep", [128, OD], f32, kind="ExternalInput")
    hst_d = nc.dram_tensor("hst16", [128, HK, B * TSH], f16, kind="ExternalInput")
    # per-layer (b_ih + b_hh), gate-permuted, replicated over partitions
    bi0_d = nc.dram_tensor("bihh0", [128, 4096], f16, kind="ExternalInput")
    bi1_d = nc.dram_tensor("bihh1", [128, 4096], f16, kind="ExternalInput")

    out_d = nc.dram_tensor("out", [B * TSH * U, OD], f16, kind="ExternalOutput")

    # ---------------- internal dram ----------------
    x0_d = nc.dram_tensor("X0d", [U, B, 4096], f8)
    x1_d = nc.dram_tensor("X1d", [U, B, 4096], f8)

    LAG = 20  # layer-1 runs LAG wavefronts behind layer-0

    with tile.TileContext(nc) as tc:
        with tc.tile_pool(name="const", bufs=1) as pc:
            # constants (small, urgent loads first on the sync queue)
            eye128_sb = pc.tile([128, 128], f16, tag="eye128")
            nc.sync.dma_start(eye128_sb[:], eye128_d.ap())
            yidx_sb = pc.tile([128, B * U // 128], i32, tag="yidx")
            nc.sync.dma_start(yidx_sb[:], yidx_d.ap())
            inj_sb = pc.tile([8, 8], f8, tag="inj")
            nc.sync.dma_start(inj_sb[:], inj_d.ap())
            hst_sb = pc.tile([128, HK, B * TSH], f16, tag="hst")
            nc.scalar.dma_start(hst_sb[:], hst_d.ap())
            if with_biases:
                bi0_sb = pc.tile([128, 4096], f16, tag="bi0")
                nc.scalar.dma_start(bi0_sb[:], bi0_d.ap())
                bi1_sb = pc.tile([128, 4096], f16, tag="bi1")
                nc.scalar.dma_start(bi1_sb[:], bi1_d.ap())
            # h_dec transposed history, both layers (fp16)
            hdec = [pc.tile([128, HK, U, B], f16, tag=f"hdec{l}",
                            name=f"hdec{l}") for l in range(2)]

            pw_ctx = tc.tile_pool(name="whh", bufs=1)
            pw = pw_ctx.__enter__()
            # recurrent weights (resident for whole LSTM); per-kc chunked
            # loads on the vector queue so early chunks arrive early and
            # don't block the sync queue.
            whh_sb = [pw.tile([128, HK, NG, 1024], f16, tag=f"whh{l}",
                               name=f"whh{l}") for l in range(2)]
            # whh0 loads are emitted after x0_block(0) (same scalar queue)
            # so the startup-critical X0 weight chunks go first; whh1 loads
            # are deferred into the wavefront loop (needed from wavefront
            # LAG on).

            # ---------------- main pools (LSTM + pre phases) --------------
            with (
                tc.tile_pool(name="lstmS", bufs=1) as lS,
                tc.tile_pool(name="lstmPS", bufs=1, space="PSUM") as lP,
            ):
                # ---------------- P1: embedding gather + eys^T ------------
                sc = nc.named_scope("gather"); sc.__enter__()
                NCH = B * U // 128  # row chunks of 128
                eyst = lS.tile([128, EK, B * U], f16, tag="eyst")
                for ch in range(NCH):
                    g32 = lS.tile([128, E], f32, tag="g32", bufs=1)
                    nc.gpsimd.indirect_dma_start(
                        out=g32[:], out_offset=None, in_=embed_d.ap(),
                        in_offset=bass.IndirectOffsetOnAxis(
                            ap=yidx_sb[:, ch:ch + 1], axis=0))
                    g16 = lS.tile([128, E], f16, tag="g16", bufs=1)
                    nc.vector.tensor_copy(g16[:], g32[:])
                    for ec in range(EK):
                        tp = lP.tile([128, 128], f16, tag="tp128", bufs=2)
                        nc.tensor.transpose(
                            tp[:], g16[:, ec * 128:(ec + 1) * 128], eye128_sb[:])
                        nc.vector.tensor_copy(
                            eyst[:, ec, ch * 128:(ch + 1) * 128], tp[:])
                sc.__exit__(None, None, None)

                gate_ps = [lP.tile([128, 1024], f32, tag=f"gates{l}",
                                   name=f"gates{l}") for l in range(2)]
                nc.vector.memset(gate_ps[0][:], 0.0)
                nc.vector.memset(gate_ps[1][:], 0.0)
                czero = [lS.tile([128, 256], f32, tag=f"c{l}", name=f"cz{l}",
                                  bufs=2) for l in range(2)]
                nc.gpsimd.memset(czero[0][:], 0.0)
                nc.gpsimd.memset(czero[1][:], 0.0)
                cprev = [czero[0], czero[1]]
                xsrc = [x0_d, x1_d]

                # ---------------- P2: X0 u-blocks -------------------------
                def x0_block(g):
                    # X0[u-block g] = eys-block @ W_ih0^T, streamed weights
                    for nc_ in range(8):
                        w0c = lS.tile([128, EK, 512], f16, tag="w0c", bufs=2)
                        eng = nc.sync if nc_ % 2 == 0 else nc.scalar
                        eng.dma_start(
                            w0c[:],
                            wih0_d.ap()[:, :, nc_ * 512:(nc_ + 1) * 512])
                        ps = lP.tile([128, 512], f32, tag="xps", bufs=2)
                        for ec in range(EK):
                            nc.tensor.matmul(
                                ps[:],
                                eyst[:, ec, g * 128:(g + 1) * 128],
                                w0c[:, ec, :],
                                start=(ec == 0), stop=(ec == EK - 1))
                        x0c = lS.tile([128, 512], f8, tag="x0c", bufs=2)
                        if with_biases:
                            nc.vector.tensor_add(
                                x0c[:], ps[:],
                                bi0_sb[:, nc_ * 512:(nc_ + 1) * 512])
                        else:
                            nc.vector.tensor_copy(x0c[:], ps[:])
                        nc.sync.dma_start(
                            x0_d.ap()[g * 16:(g + 1) * 16, :,
                                      nc_ * 512:(nc_ + 1) * 512],
                            x0c[:])

                def x0_rest(nc_):
                    # one W_ih0 chunk, X0 for u-blocks 1..3 (loads W once)
                    w0c = lS.tile([128, EK, 512], f16, tag="w0c", bufs=2)
                    eng = nc.sync if nc_ % 2 == 0 else nc.scalar
                    eng.dma_start(
                        w0c[:], wih0_d.ap()[:, :, nc_ * 512:(nc_ + 1) * 512])
                    for g in range(1, UG):
                        ps = lP.tile([128, 512], f32, tag="xps", bufs=2)
                        for ec in range(EK):
                            nc.tensor.matmul(
                                ps[:],
                                eyst[:, ec, g * 128:(g + 1) * 128],
                                w0c[:, ec, :],
                                start=(ec == 0), stop=(ec == EK - 1))
                        x0c = lS.tile([128, 512], f8, tag="x0c", bufs=2)
                        if with_biases:
                            nc.vector.tensor_add(
                                x0c[:], ps[:],
                                bi0_sb[:, nc_ * 512:(nc_ + 1) * 512])
                        else:
                            nc.vector.tensor_copy(x0c[:], ps[:])
                        nc.sync.dma_start(
                            x0_d.ap()[g * 16:(g + 1) * 16, :,
                                      nc_ * 512:(nc_ + 1) * 512],
                            x0c[:])

                # ---------------- P3: LSTM pieces -------------------------
                def lstm_rec(l, u):
                    # inject + recurrent matmuls into gates psum [PE].
                    # inject first: it has no h-dependency, so it can run
                    # during the previous step's activation chain.
                    pg = gate_ps[l]
                    xf = lS.tile([8, 4096], f8, tag="xf", bufs=2)
                    nc.gpsimd.dma_start(xf[:], xsrc[l].ap()[u])
                    for hf in range(2):
                        sl = slice(hf * 512, (hf + 1) * 512)
                        for j in range(NG):
                            nc.tensor.matmul(
                                pg[32 * j:32 * j + 8, sl], inj_sb[:],
                                xf[:, j * 1024 + hf * 512:
                                   j * 1024 + (hf + 1) * 512],
                                tile_position=(0, 32 * j),
                                start=True, stop=(u == 0))
                    if u > 0:
                        for kc in range(HK):
                            for hf in range(2):
                                sl = slice(hf * 512, (hf + 1) * 512)
                                for j in range(NG):
                                    nc.tensor.matmul(
                                        pg[32 * j:32 * j + 8, sl],
                                        hdec[l][:, kc, u - 1, :],
                                        whh_sb[l][:, kc, j,
                                                  hf * 512:(hf + 1) * 512],
                                        tile_position=(0, 32 * j),
                                        start=False, stop=(kc == HK - 1))

                hbuf = [None, None]  # last h tile per layer

                def lstm_chain(l, u, solo=False):
                    # gates -> sigmoid/tanh -> c,h  [ACT + DVE only]
                    # sigmoid split i,f | o so the c-path starts ~0.4us
                    # earlier.
                    pg = gate_ps[l]
                    sig = lS.tile([128, 768], f16, tag=f"sig{l}")
                    nc.scalar.activation(sig[:, 0:512], pg[:, 0:512],
                                         AF.Sigmoid)
                    tg = lS.tile([128, 256], f16, tag=f"tg{l}")
                    nc.scalar.activation(tg[:], pg[:, 768:1024], AF.Tanh)
                    nc.scalar.activation(sig[:, 512:768], pg[:, 512:768],
                                         AF.Sigmoid)
                    cnew = lS.tile([128, 256], f32, tag=f"c{l}", bufs=2)
                    nc.vector.tensor_mul(cnew[:], sig[:, 256:512], cprev[l][:])
                    t1 = lS.tile([128, 256], f32, tag=f"t1{l}")
                    nc.vector.tensor_mul(t1[:], sig[:, 0:256], tg[:])
                    nc.vector.tensor_add(cnew[:], cnew[:], t1[:])
                    cprev[l] = cnew
                    tc_ = lS.tile([128, 256], f16, tag=f"tc{l}")
                    nc.scalar.activation(tc_[:], cnew[:], AF.Tanh)
                    h = lS.tile([128, 256], f16, tag=f"h{l}", bufs=2)
                    nc.vector.tensor_mul(h[:], sig[:, 512:768], tc_[:])
                    hbuf[l] = h

                def lstm_transp(l, u):
                    # h -> hdec[l][:, :, u, :]  [PE transpose + DVE copy]
                    h = hbuf[l]
                    for cb in range(2):
                        tp = lP.tile([128, 128], f16, tag="tp128", bufs=2)
                        nc.tensor.transpose(
                            tp[:], h[:, cb * 128:(cb + 1) * 128],
                            eye128_sb[:])
                        hd = hdec[l][:, 0, u, :]  # (128, B) at kc=0
                        dst = bass.AP(hd.tensor, hd.offset + cb * U * B,
                                      [hd.ap[0], [2 * U * B, NG], [1, B]])
                        src_ap = bass.AP(tp[:].tensor, tp[:].offset,
                                         [tp[:].ap[0], [32, NG], [1, B]])
                        nc.vector.tensor_copy(dst, src_ap)

                def x1_subblock(kb, sb):
                    # 4 of 16 weight chunks of X1 u-block kb; loads alternate
                    # sync/scalar DMA queues to double effective bandwidth
                    hd0 = hdec[0]
                    for nc2 in range(4 * sb, 4 * sb + 4):
                        w1c = lS.tile([128, HK, 256], f16, tag="w1c", bufs=2)
                        eng = nc.sync if nc2 % 2 == 0 else nc.scalar
                        eng.dma_start(w1c[:], wih1_d.ap()[nc2])
                        ps = lP.tile([128, 512], f32, tag="xps", bufs=2)
                        for kc in range(HK):
                            nc.tensor.matmul(
                                ps[:, 0:256],
                                hd0[:, kc, kb * 16:(kb + 1) * 16, :],
                                w1c[:, kc, :],
                                start=(kc == 0), stop=(kc == HK - 1))
                        x1c = lS.tile([128, 256], f8, tag="x1c", bufs=2)
                        if with_biases:
                            nc.vector.tensor_add(
                                x1c[:], ps[:, 0:256],
                                bi1_sb[:, nc2 * 256:(nc2 + 1) * 256])
                        else:
                            nc.vector.tensor_copy(x1c[:], ps[:, 0:256])
                        nc.sync.dma_start(
                            x1_d.ap()[kb * 16:(kb + 1) * 16, :,
                                      nc2 * 256:(nc2 + 1) * 256],
                            x1c[:])

                # early ze: encoder projection needs only hst + wenc;
                # computed during the l1-solo tail to fill PE idle there
                benc_sb = lS.tile([128, JC], f32, tag="bencs")
                ze_sb = lS.tile([128, JC, B * TSH], f16, tag="ze")

                def ze_compute():
                    nc.sync.dma_start(benc_sb[:], benc_d.ap())
                    for jc in range(JC):
                        wencc = lS.tile([128, HK, 128], f16, tag="wencc",
                                        bufs=1)
                        eng = nc.sync if jc % 2 == 0 else nc.scalar
                        eng.dma_start(wencc[:], wenc_d.ap()[:, :, jc, :])
                        zp = lP.tile([128, 512], f32, tag="xps", bufs=2)
                        for ec in range(HK):
                            nc.tensor.matmul(zp[:, 0:128],
                                             wencc[:, ec, :],
                                             hst_sb[:, ec, :],
                                             start=(ec == 0),
                                             stop=(ec == HK - 1))
                        nc.scalar.activation(ze_sb[:, jc, :], zp[:, 0:128],
                                             AF.Identity,
                                             bias=benc_sb[:, jc:jc + 1])

                # ---------------- wavefront schedule ----------------------
                # per wavefront w:
                #   PE:  T(l0,w-1) R(l0,w) T(l1,w-1-LAG) R(l1,w-LAG) [x1blk]
                #   chains emitted after all PE work so neither layer's
                #   ACT/DVE ops gate the other layer's PE stream.
                with nc.named_scope("x0b0"):
                    x0_block(0)
                for kc in range(HK):
                    eng = nc.sync if kc % 2 == 0 else nc.scalar
                    eng.dma_start(whh_sb[0][:, kc, :, :],
                                  whh0_d.ap()[:, kc, :, :])
                for w in range(U + LAG + 1):
                    u0, u1 = w, w - LAG
                    with nc.named_scope(f"w{w:02d}"):
                        if 1 <= u0 <= U:
                            lstm_transp(0, u0 - 1)
                        if u0 < U:
                            lstm_rec(0, u0)
                        if 1 <= u1 <= U:
                            lstm_transp(1, u1 - 1)
                        if 0 <= u1 < U:
                            lstm_rec(1, u1)
                        dual = (u0 < U) and (0 <= u1 < U)
                        if u0 < U:
                            lstm_chain(0, u0, solo=not dual)
                        if 0 <= u1 < U:
                            lstm_chain(1, u1, solo=not dual)
                    if 1 <= u0 <= 8:
                        with nc.named_scope(f"x0r{u0 - 1}"):
                            x0_rest(u0 - 1)
                    if u0 == 10:
                        for kc in range(HK):
                            nc.scalar.dma_start(whh_sb[1][:, kc, :, :],
                                                whh1_d.ap()[:, kc, :, :])
                    if u0 == U + 2:
                        with nc.named_scope("ze"):
                            ze_compute()
                    if 16 <= u0 < 68 and (u0 - 16) % 16 < 4:
                        kb, sb = (u0 - 16) // 16, (u0 - 16) % 16
                        with nc.named_scope(f"x1b{kb}_{sb}"):
                            x1_subblock(kb, sb)

            pw_ctx.__exit__(None, None, None)

            # ---------------- P4/P5: joint ----------------
            sc_j = nc.named_scope("joint"); sc_j.__enter__()
            with (
                tc.tile_pool(name="jS", bufs=1) as jS,
                tc.tile_pool(name="jPS", bufs=1, space="PSUM") as jP,
            ):
                wdec_sb = jS.tile([128, HK, JC, 128], f16, tag="wdec")
                nc.sync.dma_start(wdec_sb[:], wdec_d.ap())
                wout_sb = jS.tile([128, JC, OD], f16, tag="woutr")
                nc.sync.dma_start(wout_sb[:], wout_d.ap())
                if with_out_bias:
                    bout_sb = jS.tile([128, OD], f32, tag="bouts")
                    nc.sync.dma_start(bout_sb[:], bout_d.ap())

                # ze (J, b*tl) and zd (J, b, u)
                sc_z = nc.named_scope("zedzd"); sc_z.__enter__()
                zd_sb = jS.tile([128, JC, U, B], f16, tag="zd")
                for jc in range(JC):
                    zp = jP.tile([128, U * B], f32, tag="zdps", bufs=2)
                    for kc in range(HK):
                        nc.tensor.matmul(
                            zp[:], wdec_sb[:, kc, jc, :],
                            hdec[1][:, kc, :, :].rearrange("p u b -> p (u b)"),
                            start=(kc == 0), stop=(kc == HK - 1))
                    nc.vector.tensor_copy(
                        zd_sb[:, jc, :, :].rearrange("p u b -> p (u b)"), zp[:])

                sc_z.__exit__(None, None, None)
                sc_b = nc.named_scope("jblk"); sc_b.__enter__()
                # joint blocks: 128 rows = 2 (b,tl) pairs x U
                n_pairs = B * TSH
                rows_per_pair = U
                ppb = 128 // rows_per_pair  # pairs per block
                BTSH = B * TSH
                for blk in range(n_pairs // ppb):
                    pr0 = blk * ppb
                    b = pr0 // TSH
                    zjt = jS.tile([128, JC, 128], f16, tag="zjt", bufs=2)
                    zj = jS.tile([128, JC, 128], f16, tag="zj", bufs=2)
                    # fused over all jc: out[p, jc, a, u] = ze[p, jc, pr0+a]
                    #                                     + zd[p, jc, u, b]
                    zjt_ap = zjt[:, :, :].rearrange(
                        "p jc (a u) -> p jc a u", a=ppb)
                    zea = ze_sb[:, 0, 0]
                    ze_bc = bass.AP(zea.tensor, zea.offset + pr0,
                                    [zea.ap[0], [BTSH, JC], [1, ppb], [0, U]])
                    zda = zd_sb[:, 0, 0, 0]
                    zd_bc = bass.AP(zda.tensor, zda.offset + b,
                                    [zda.ap[0], [U * B, JC], [0, ppb], [B, U]])
                    nc.vector.tensor_tensor(
                        zjt_ap, ze_bc, zd_bc, op=mybir.AluOpType.add)
                    nc.scalar.activation(
                        zj[:, :, :].rearrange("p jc m -> p (jc m)"),
                        zjt[:, :, :].rearrange("p jc m -> p (jc m)"), AF.Tanh)
                    ops_ = jP.tile([128, OD], f32, tag="outps", bufs=2)
                    for n2 in range(2):
                        for jc in range(JC):
                            nc.tensor.matmul(
                                ops_[:, n2 * 512:(n2 + 1) * 512],
                                zj[:, jc, :],
                                wout_sb[:, jc, n2 * 512:(n2 + 1) * 512],
                                start=(jc == 0), stop=(jc == JC - 1))
                    osb = jS.tile([128, OD], f16, tag="osb", bufs=3)
                    if with_out_bias:
                        nc.vector.tensor_add(osb[:], ops_[:], bout_sb[:])
                    else:
                        # split the psum->sbuf evacuation across DVE + ACT
                        # so neither engine bottlenecks the block pipeline
                        nc.vector.tensor_copy(osb[:, 0:512], ops_[:, 0:512])
                        nc.scalar.activation(osb[:, 512:1024],
                                             ops_[:, 512:1024], AF.Identity)
                    eng = nc.sync if blk % 2 == 0 else nc.scalar
                    eng.dma_start(
                        out_d.ap()[blk * 128:(blk + 1) * 128, :], osb[:])
                sc_b.__exit__(None, None, None)
            sc_j.__exit__(None, None, None)

    nc.compile()
    return nc


# ---------------- host-side prep ----------------

def gate_perm():
    """perm[j*1024 + s] -> row index in torch (i,f,g,o) 4H gate layout,
    with group-local order [i|f|o|g]."""
    perm = np.zeros(4 * H, dtype=np.int64)
    for j in range(NG):
        base = j * 1024
        hid = np.arange(256) + j * 256
        perm[base + 0:base + 256] = 0 * H + hid      # i
        perm[base + 256:base + 512] = 1 * H + hid    # f
        perm[base + 512:base + 768] = 3 * H + hid    # o
        perm[base + 768:base + 1024] = 2 * H + hid   # g
    return perm


def prep_inputs(hs_pad, ys_in_pad, embed, W_ih0, W_hh0, b_ih0, b_hh0,
                W_ih1, W_hh1, b_ih1, b_hh1, W_enc, b_enc, W_dec, W_out, b_out,
                U=64, n_cores=8):
    f16 = ml_dtypes.float16 if not hasattr(np, "float16") else np.float16
    perm = gate_perm()

    def wiht(W, KD, KC):  # (4H, KD) -> (128, KC, 4096) fp16, permuted gates
        Wp = W[perm]                      # (4096, KD)
        return np.ascontiguousarray(
            Wp.T.reshape(KC, 128, 4096).transpose(1, 0, 2)).astype(np.float16)

    def whht(W):  # (4H, H) -> (128, HK, NG, 1024) fp16
        Wp = W[perm]                      # (4096, 1024) rows=permuted gates
        # [p, kc, j, n] = Wp[j*1024+n, kc*128+p]
        a = Wp.T.reshape(HK, 128, NG, 1024).transpose(1, 0, 2, 3)
        return np.ascontiguousarray(a).astype(np.float16)

    ins = {}
    ins["embed"] = np.asarray(embed, np.float32)
    ys = np.asarray(ys_in_pad).astype(np.int32)   # (B, U)
    NCH = B * U // 128
    yy = np.zeros((128, NCH), np.int32)
    for ch in range(NCH):
        p = np.arange(128)
        yy[:, ch] = ys[p % 8, ch * 16 + p // 8]
    ins["yidx"] = yy
    ins["wih0t"] = wiht(W_ih0, E, EK)
    w1 = wiht(W_ih1, H, HK)  # (128, HK, 4096)
    ins["wih1t"] = np.ascontiguousarray(
        w1.reshape(128, HK, 16, 256).transpose(2, 0, 1, 3))
    ins["whh0t"] = whht(W_hh0)
    ins["whh1t"] = whht(W_hh1)
    ins["inj8"] = np.eye(8).astype(ml_dtypes.float8_e4m3)
    ins["eye128"] = np.eye(128, dtype=np.float16)
    # [p, ec, jc, m] = W[jc*128+m, ec*128+p]
    def wjt(W, KC):
        a = W.T.reshape(KC, 128, JC, 128).transpose(1, 0, 2, 3)
        return np.ascontiguousarray(a).astype(np.float16)
    ins["wenct"] = wjt(W_enc, HK)
    ins["wdect"] = wjt(W_dec, HK)
    # [p, jc, od] = W_out[od, jc*128+p]
    ins["woutt"] = np.ascontiguousarray(
        W_out.T.reshape(JC, 128, OD).transpose(1, 0, 2)).astype(np.float16)
    ins["benc"] = np.ascontiguousarray(
        b_enc.reshape(JC, 128).T).astype(np.float32)
    ins["boutrep"] = np.tile(np.asarray(b_out, np.float32)[None, :], (128, 1))
    ins["bihh0"] = np.tile(((b_ih0 + b_hh0)[perm]).astype(np.float16)[None, :],
                           (128, 1))
    ins["bihh1"] = np.tile(((b_ih1 + b_hh1)[perm]).astype(np.float16)[None, :],
                           (128, 1))

    maps = []
    for c in range(n_cores):
        m = dict(ins)
        # [p, ec, r] = hs[b, TSH*c + tl, ec*128+p], r = b*TSH+tl
        sl = np.asarray(hs_pad[:, TSH * c:TSH * (c + 1), :], np.float32)
        a = sl.reshape(B * TSH, HK, 128).transpose(2, 1, 0)
        m["hst16"] = np.ascontiguousarray(a).astype(np.float16)
        maps.append(m)
    return maps


def gather_output(results):
    outs = [np.asarray(r["out"], np.float32).reshape(B, TSH, -1, OD)
            for r in results]
    return np.concatenate(outs, axis=1)


# ---------------- entry point ----------------
import sys as _sys
import types as _types

# Recreate the missing antenv.axon_hooks so trace=True works under axon
# (used only when BASS_TRACE=1 is set by a profiling harness).
if "antenv.axon_hooks" not in _sys.modules:
    _m = _types.ModuleType("antenv.axon_hooks")

    def _get_hook():
        try:
            from trn_agent_boot.trn_boot import _ntff_profile_via_ctypes
            return _ntff_profile_via_ctypes("/opt/axon/libaxon_pjrt.so")
        except Exception:
            return None
    _m.get_axon_ntff_profile_hook = _get_hook
    _sys.modules["antenv.axon_hooks"] = _m

_NC = None
last_results = None


def kernel(**inputs):
    """Full-input RNN-T decoder: returns (B, T, U, ODIM) float32."""
    global _NC, last_results
    from concourse.bass_utils import run_bass_kernel_spmd
    U = int(np.asarray(inputs["ys_in_pad"]).shape[1])
    wb = any(float(np.abs(np.asarray(inputs[k])).max()) != 0.0
             for k in ("b_ih0", "b_hh0", "b_ih1", "b_hh1"))
    wob = float(np.abs(np.asarray(inputs["b_out"])).max()) != 0.0
    if _NC is None:
        _NC = build_program(U=U, n_cores=8, with_biases=wb, with_out_bias=wob)
    maps = prep_inputs(**inputs, U=U)
    res = run_bass_kernel_spmd(_NC, maps, core_ids=list(range(8)))
    last_results = res
    return gather_output(res.results)



# revision 35
# speedup vs baseline: 1.2150x; 1.0002x over previous
"""RNN-T decoder kernel for TRN2 (8 cores, T-sharded joint, replicated LSTM).

Layout notes
------------
B=8, T=128, U=64, E=512, H=1024 (8 k-chunks), J=640 (5 j-chunks), OD=1024.
Each core handles T-slice [16c, 16c+16) of the joint; the 2-layer LSTM over U
is computed identically (replicated, all 8 batches) on every core.

Gate permutation: hidden dim is split in 4 quarters (col-tile groups). Group
j's 1024 gate columns are [i_j | f_j | o_j | g_j] (256 each), where x_j acts
on hidden units [256j, 256j+256). Weights/bias/X tensors are host-permuted
to this order.

Gates PSUM tile (128, 1024): group j occupies partitions [32j, 32j+8)
(batch-major), accumulated by 4-way column-packed matmuls (tile_position).

Schedule: software-pipelined wavefronts with layer-1 lagging LAG=18 steps.
Per wavefront the PE runs [T(l0,u-1), R(l0,u), T(l1,v-1), R(l1,v)] back to
back; each layer's activation chain (ACT+DVE) is emitted after all PE work
so neither layer's chain gates the other's stream. X0 is produced in
u-blocks ahead of consumption (block 0 up front, the rest one weight-chunk
per early wavefront); X1 blocks are split in 4-chunk sub-blocks spread over
2 wavefronts with sync/scalar dual-queue fp8 weight loads. h0 history is
kept in both f16 (recurrence) and fp8 (X1 matmuls). The joint runs after
the LSTM with fused broadcast-add/tanh over all jc and f16 weights/output.
"""
import numpy as np
import ml_dtypes

import concourse.bass as bass
import concourse.bacc as bacc
import concourse.mybir as mybir
import concourse.tile as tile

dt = mybir.dt
AF = mybir.ActivationFunctionType

B, T, E, H, J, OD = 8, 128, 512, 1024, 640, 1024
HK = H // 128   # 8 h-chunks
JC = J // 128   # 5 j-chunks
EK = E // 128   # 4 e-chunks
TSH = T // 8    # 16 t per core
NG = 4          # col-tile groups


def bcast_mid(ap, count):
    """(128, N) AP -> (128, count, N) with a 0-step middle dim."""
    return bass.AP(ap.tensor, ap.offset, [ap.ap[0], [0, count], ap.ap[1]])


def build_program(U=64, n_cores=8, with_biases=False,
                  with_out_bias=False):
    nc = bacc.Bacc("TRN2", target_bir_lowering=False, debug=False,
                   num_devices=n_cores)
    f16, f32, f32r, i32 = dt.float16, dt.float32, dt.float32r, dt.int32
    UG = U // 16  # u-blocks of 16
    assert U % 16 == 0

    # ---------------- external inputs ----------------
    embed_d = nc.dram_tensor("embed", [OD, E], f32, kind="ExternalInput")
    yidx_d = nc.dram_tensor("yidx", [128, B * U // 128], i32, kind="ExternalInput")
    wih0_d = nc.dram_tensor("wih0t", [128, EK, 4096], f16, kind="ExternalInput")
    wih1_d = nc.dram_tensor("wih1t", [16, 128, HK, 256], f16, kind="ExternalInput")
    whh0_d = nc.dram_tensor("whh0t", [128, HK, NG, 1024], f16, kind="ExternalInput")
    whh1_d = nc.dram_tensor("whh1t", [128, HK, NG, 1024], f16, kind="ExternalInput")
    inj_d = nc.dram_tensor("inj8", [8, 8], f8, kind="ExternalInput")
    eye128_d = nc.dram_tensor("eye128", [128, 128], f16, kind="ExternalInput")
    wenc_d = nc.dram_tensor("wenct", [128, HK, JC, 128], f16, kind="ExternalInput")
    wdec_d = nc.dram_tensor("wdect", [128, HK, JC, 128], f16, kind="ExternalInput")
    wout_d = nc.dram_tensor("woutt", [128, JC, OD], f16, kind="ExternalInput")
    benc_d = nc.dram_tensor("benc", [128, JC], f32, kind="ExternalInput")
    bout_d = nc.dram_tensor("boutrep", [128, OD], f32, kind="ExternalInput")
    hst_d = nc.dram_tensor("hst16", [128, HK, B * TSH], f16, kind="ExternalInput")
    # per-layer (b_ih + b_hh), gate-permuted, replicated over partitions
    bi0_d = nc.dram_tensor("bihh0", [128, 4096], f16, kind="ExternalInput")
    bi1_d = nc.dram_tensor("bihh1", [128, 4096], f16, kind="ExternalInput")

    out_d = nc.dram_tensor("out", [B * TSH * U, OD], f16, kind="ExternalOutput")

    # ---------------- internal dram ----------------
    x0_d = nc.dram_tensor("X0d", [U, B, 4096], f8)
    x1_d = nc.dram_tensor("X1d", [U, B, 4096], f8)

    LAG = 20  # layer-1 runs LAG wavefronts behind layer-0

    with tile.TileContext(nc) as tc:
        with tc.tile_pool(name="const", bufs=1) as pc:
            # constants (small, urgent loads first on the sync queue)
            eye128_sb = pc.tile([128, 128], f16, tag="eye128")
            nc.sync.dma_start(eye128_sb[:], eye128_d.ap())
            yidx_sb = pc.tile([128, B * U // 128], i32, tag="yidx")
            nc.sync.dma_start(yidx_sb[:], yidx_d.ap())
            inj_sb = pc.tile([8, 8], f8, tag="inj")
            nc.sync.dma_start(inj_sb[:], inj_d.ap())
            hst_sb = pc.tile([128, HK, B * TSH], f16, tag="hst")
            nc.scalar.dma_start(hst_sb[:], hst_d.ap())
            if with_biases:
                bi0_sb = pc.tile([128, 4096], f16, tag="bi0")
                nc.scalar.dma_start(bi0_sb[:], bi0_d.ap())
                bi1_sb = pc.tile([128, 4096], f16, tag="bi1")
                nc.scalar.dma_start(bi1_sb[:], bi1_d.ap())
            # h_dec transposed history, both layers (fp16)
            hdec = [pc.tile([128, HK, U, B], f16, tag=f"hdec{l}",
                            name=f"hdec{l}") for l in range(2)]

            pw_ctx = tc.tile_pool(name="whh", bufs=1)
            pw = pw_ctx.__enter__()
            # recurrent weights (resident for whole LSTM); per-kc chunked
            # loads on the vector queue so early chunks arrive early and
            # don't block the sync queue.
            whh_sb = [pw.tile([128, HK, NG, 1024], f16, tag=f"whh{l}",
                               name=f"whh{l}") for l in range(2)]
            # whh0 loads are emitted after x0_block(0) (same scalar queue)
            # so the startup-critical X0 weight chunks go first; whh1 loads
            # are deferred into the wavefront loop (needed from wavefront
            # LAG on).

            # ---------------- main pools (LSTM + pre phases) --------------
            with (
                tc.tile_pool(name="lstmS", bufs=1) as lS,
                tc.tile_pool(name="lstmPS", bufs=1, space="PSUM") as lP,
            ):
                # ---------------- P1: embedding gather + eys^T ------------
                sc = nc.named_scope("gather"); sc.__enter__()
                NCH = B * U // 128  # row chunks of 128
                eyst = lS.tile([128, EK, B * U], f16, tag="eyst")
                for ch in range(NCH):
                    g32 = lS.tile([128, E], f32, tag="g32", bufs=1)
                    nc.gpsimd.indirect_dma_start(
                        out=g32[:], out_offset=None, in_=embed_d.ap(),
                        in_offset=bass.IndirectOffsetOnAxis(
                            ap=yidx_sb[:, ch:ch + 1], axis=0))
                    g16 = lS.tile([128, E], f16, tag="g16", bufs=1)
                    nc.vector.tensor_copy(g16[:], g32[:])
                    for ec in range(EK):
                        tp = lP.tile([128, 128], f16, tag="tp128", bufs=2)
                        nc.tensor.transpose(
                            tp[:], g16[:, ec * 128:(ec + 1) * 128], eye128_sb[:])
                        nc.vector.tensor_copy(
                            eyst[:, ec, ch * 128:(ch + 1) * 128], tp[:])
                sc.__exit__(None, None, None)

                gate_ps = [lP.tile([128, 1024], f32, tag=f"gates{l}",
                                   name=f"gates{l}") for l in range(2)]
                nc.vector.memset(gate_ps[0][:], 0.0)
                nc.vector.memset(gate_ps[1][:], 0.0)
                czero = [lS.tile([128, 256], f32, tag=f"c{l}", name=f"cz{l}",
                                  bufs=2) for l in range(2)]
                nc.gpsimd.memset(czero[0][:], 0.0)
                nc.gpsimd.memset(czero[1][:], 0.0)
                cprev = [czero[0], czero[1]]
                xsrc = [x0_d, x1_d]

                # ---------------- P2: X0 u-blocks -------------------------
                def x0_block(g):
                    # X0[u-block g] = eys-block @ W_ih0^T, streamed weights
                    for nc_ in range(8):
                        w0c = lS.tile([128, EK, 512], f16, tag="w0c", bufs=2)
                        eng = nc.sync if nc_ % 2 == 0 else nc.scalar
                        eng.dma_start(
                            w0c[:],
                            wih0_d.ap()[:, :, nc_ * 512:(nc_ + 1) * 512])
                        ps = lP.tile([128, 512], f32, tag="xps", bufs=2)
                        for ec in range(EK):
                            nc.tensor.matmul(
                                ps[:],
                                eyst[:, ec, g * 128:(g + 1) * 128],
                                w0c[:, ec, :],
                                start=(ec == 0), stop=(ec == EK - 1))
                        x0c = lS.tile([128, 512], f8, tag="x0c", bufs=2)
                        if with_biases:
                            nc.vector.tensor_add(
                                x0c[:], ps[:],
                                bi0_sb[:, nc_ * 512:(nc_ + 1) * 512])
                        else:
                            nc.vector.tensor_copy(x0c[:], ps[:])
                        nc.sync.dma_start(
                            x0_d.ap()[g * 16:(g + 1) * 16, :,
                                      nc_ * 512:(nc_ + 1) * 512],
                            x0c[:])

                def x0_rest(nc_):
                    # one W_ih0 chunk, X0 for u-blocks 1..3 (loads W once)
                    w0c = lS.tile([128, EK, 512], f16, tag="w0c", bufs=2)
                    eng = nc.sync if nc_ % 2 == 0 else nc.scalar
                    eng.dma_start(
                        w0c[:], wih0_d.ap()[:, :, nc_ * 512:(nc_ + 1) * 512])
                    for g in range(1, UG):
                        ps = lP.tile([128, 512], f32, tag="xps", bufs=2)
                        for ec in range(EK):
                            nc.tensor.matmul(
                                ps[:],
                                eyst[:, ec, g * 128:(g + 1) * 128],
                                w0c[:, ec, :],
                                start=(ec == 0), stop=(ec == EK - 1))
                        x0c = lS.tile([128, 512], f8, tag="x0c", bufs=2)
                        if with_biases:
                            nc.vector.tensor_add(
                                x0c[:], ps[:],
                                bi0_sb[:, nc_ * 512:(nc_ + 1) * 512])
                        else:
                            nc.vector.tensor_copy(x0c[:], ps[:])
                        nc.sync.dma_start(
                            x0_d.ap()[g * 16:(g + 1) * 16, :,
                                      nc_ * 512:(nc_ + 1) * 512],
                            x0c[:])

                # ---------------- P3: LSTM pieces -------------------------
                def lstm_rec(l, u):
                    # inject + recurrent matmuls into gates psum [PE].
                    # inject first: it has no h-dependency, so it can run
                    # during the previous step's activation chain.
                    pg = gate_ps[l]
                    xf = lS.tile([8, 4096], f8, tag="xf", bufs=2)
                    nc.gpsimd.dma_start(xf[:], xsrc[l].ap()[u])
                    for hf in range(2):
                        sl = slice(hf * 512, (hf + 1) * 512)
                        for j in range(NG):
                            nc.tensor.matmul(
                                pg[32 * j:32 * j + 8, sl], inj_sb[:],
                                xf[:, j * 1024 + hf * 512:
                                   j * 1024 + (hf + 1) * 512],
                                tile_position=(0, 32 * j),
                                start=True, stop=(u == 0))
                    if u > 0:
                        for kc in range(HK):
                            for hf in range(2):
                                sl = slice(hf * 512, (hf + 1) * 512)
                                for j in range(NG):
                                    nc.tensor.matmul(
                                        pg[32 * j:32 * j + 8, sl],
                                        hdec[l][:, kc, u - 1, :],
                                        whh_sb[l][:, kc, j,
                                                  hf * 512:(hf + 1) * 512],
                                        tile_position=(0, 32 * j),
                                        start=False, stop=(kc == HK - 1))

                hbuf = [None, None]  # last h tile per layer

                def lstm_chain(l, u, solo=False):
                    # gates -> sigmoid/tanh -> c,h  [ACT + DVE only]
                    # sigmoid split i,f | o so the c-path starts ~0.4us
                    # earlier.
                    pg = gate_ps[l]
                    sig = lS.tile([128, 768], f16, tag=f"sig{l}")
                    nc.scalar.activation(sig[:, 0:512], pg[:, 0:512],
                                         AF.Sigmoid)
                    tg = lS.tile([128, 256], f16, tag=f"tg{l}")
                    nc.scalar.activation(tg[:], pg[:, 768:1024], AF.Tanh)
                    nc.scalar.activation(sig[:, 512:768], pg[:, 512:768],
                                         AF.Sigmoid)
                    cnew = lS.tile([128, 256], f32, tag=f"c{l}", bufs=2)
                    nc.vector.tensor_mul(cnew[:], sig[:, 256:512], cprev[l][:])
                    t1 = lS.tile([128, 256], f32, tag=f"t1{l}")
                    nc.vector.tensor_mul(t1[:], sig[:, 0:256], tg[:])
                    nc.vector.tensor_add(cnew[:], cnew[:], t1[:])
                    cprev[l] = cnew
                    tc_ = lS.tile([128, 256], f16, tag=f"tc{l}")
                    nc.scalar.activation(tc_[:], cnew[:], AF.Tanh)
                    h = lS.tile([128, 256], f16, tag=f"h{l}", bufs=2)
                    nc.vector.tensor_mul(h[:], sig[:, 512:768], tc_[:])
                    hbuf[l] = h

                def lstm_transp(l, u):
                    # h -> hdec[l][:, :, u, :]  [PE transpose + DVE copy]
                    h = hbuf[l]
                    for cb in range(2):
                        tp = lP.tile([128, 128], f16, tag="tp128", bufs=2)
                        nc.tensor.transpose(
                            tp[:], h[:, cb * 128:(cb + 1) * 128],
                            eye128_sb[:])
                        hd = hdec[l][:, 0, u, :]  # (128, B) at kc=0
                        dst = bass.AP(hd.tensor, hd.offset + cb * U * B,
                                      [hd.ap[0], [2 * U * B, NG], [1, B]])
                        src_ap = bass.AP(tp[:].tensor, tp[:].offset,
                                         [tp[:].ap[0], [32, NG], [1, B]])
                        nc.vector.tensor_copy(dst, src_ap)

                def x1_subblock(kb, sb):
                    # 4 of 16 weight chunks of X1 u-block kb; loads alternate
                    # sync/scalar DMA queues to double effective bandwidth
                    hd0 = hdec[0]
                    for nc2 in range(4 * sb, 4 * sb + 4):
                        w1c = lS.tile([128, HK, 256], f16, tag="w1c", bufs=2)
                        eng = nc.sync if nc2 % 2 == 0 else nc.scalar
                        eng.dma_start(w1c[:], wih1_d.ap()[nc2])
                        ps = lP.tile([128, 512], f32, tag="xps", bufs=2)
                        for kc in range(HK):
                            nc.tensor.matmul(
                                ps[:, 0:256],
                                hd0[:, kc, kb * 16:(kb + 1) * 16, :],
                                w1c[:, kc, :],
                                start=(kc == 0), stop=(kc == HK - 1))
                        x1c = lS.tile([128, 256], f8, tag="x1c", bufs=2)
                        if with_biases:
                            nc.vector.tensor_add(
                                x1c[:], ps[:, 0:256],
                                bi1_sb[:, nc2 * 256:(nc2 + 1) * 256])
                        else:
                            nc.vector.tensor_copy(x1c[:], ps[:, 0:256])
                        nc.sync.dma_start(
                            x1_d.ap()[kb * 16:(kb + 1) * 16, :,
                                      nc2 * 256:(nc2 + 1) * 256],
                            x1c[:])

                # early ze: encoder projection needs only hst + wenc;
                # computed during the l1-solo tail to fill PE idle there
                benc_sb = lS.tile([128, JC], f32, tag="bencs")
                ze_sb = lS.tile([128, JC, B * TSH], f16, tag="ze")
                zd_sb = lS.tile([128, JC, U, B], f16, tag="zd")

                def zd_chunk(c):
                    # decoder projection for u-chunk c (needs hdec1 through
                    # u=16c+15); streamed per-jc wdec tiles
                    for jc in range(JC):
                        wdecc = lS.tile([128, HK, 128], f16, tag="wdecc",
                                        bufs=2)
                        eng = nc.sync if jc % 2 == 0 else nc.scalar
                        eng.dma_start(wdecc[:], wdec_d.ap()[:, :, jc, :])
                        zp = lP.tile([128, 512], f32, tag="xps", bufs=2)
                        for kc in range(HK):
                            nc.tensor.matmul(
                                zp[:, 0:128], wdecc[:, kc, :],
                                hdec[1][:, kc, 16 * c:16 * c + 16, :]
                                .rearrange("p u b -> p (u b)"),
                                start=(kc == 0), stop=(kc == HK - 1))
                        nc.vector.tensor_copy(
                            zd_sb[:, jc, 16 * c:16 * c + 16, :]
                            .rearrange("p u b -> p (u b)"), zp[:, 0:128])

                def ze_compute():
                    nc.sync.dma_start(benc_sb[:], benc_d.ap())
                    for jc in range(JC):
                        wencc = lS.tile([128, HK, 128], f16, tag="wencc",
                                        bufs=1)
                        eng = nc.sync if jc % 2 == 0 else nc.scalar
                        eng.dma_start(wencc[:], wenc_d.ap()[:, :, jc, :])
                        zp = lP.tile([128, 512], f32, tag="xps", bufs=2)
                        for ec in range(HK):
                            nc.tensor.matmul(zp[:, 0:128],
                                             wencc[:, ec, :],
                                             hst_sb[:, ec, :],
                                             start=(ec == 0),
                                             stop=(ec == HK - 1))
                        nc.scalar.activation(ze_sb[:, jc, :], zp[:, 0:128],
                                             AF.Identity,
                                             bias=benc_sb[:, jc:jc + 1])

                # ---------------- wavefront schedule ----------------------
                # per wavefront w:
                #   PE:  T(l0,w-1) R(l0,w) T(l1,w-1-LAG) R(l1,w-LAG) [x1blk]
                #   chains emitted after all PE work so neither layer's
                #   ACT/DVE ops gate the other layer's PE stream.
                with nc.named_scope("x0b0"):
                    x0_block(0)
                for kc in range(HK):
                    eng = nc.sync if kc % 2 == 0 else nc.scalar
                    eng.dma_start(whh_sb[0][:, kc, :, :],
                                  whh0_d.ap()[:, kc, :, :])
                for w in range(U + LAG + 1):
                    u0, u1 = w, w - LAG
                    with nc.named_scope(f"w{w:02d}"):
                        if 1 <= u0 <= U:
                            lstm_transp(0, u0 - 1)
                        if u0 < U:
                            lstm_rec(0, u0)
                        if 1 <= u1 <= U:
                            lstm_transp(1, u1 - 1)
                        if 0 <= u1 < U:
                            lstm_rec(1, u1)
                        dual = (u0 < U) and (0 <= u1 < U)
                        if u0 < U:
                            lstm_chain(0, u0, solo=not dual)
                        if 0 <= u1 < U:
                            lstm_chain(1, u1, solo=not dual)
                    if 1 <= u0 <= 8:
                        with nc.named_scope(f"x0r{u0 - 1}"):
                            x0_rest(u0 - 1)
                    if u0 == 10:
                        for kc in range(HK):
                            nc.scalar.dma_start(whh_sb[1][:, kc, :, :],
                                                whh1_d.ap()[:, kc, :, :])
                    if u0 == U + 2:
                        with nc.named_scope("ze"):
                            ze_compute()
                    if u0 in (U + 3, U + 5, U + 7):
                        with nc.named_scope(f"zd{(u0 - U - 3) // 2}"):
                            zd_chunk((u0 - U - 3) // 2)
                    if 16 <= u0 < 68 and (u0 - 16) % 16 < 4:
                        kb, sb = (u0 - 16) // 16, (u0 - 16) % 16
                        with nc.named_scope(f"x1b{kb}_{sb}"):
                            x1_subblock(kb, sb)

                with nc.named_scope("zd3"):
                    zd_chunk(3)

            pw_ctx.__exit__(None, None, None)

            # ---------------- P4/P5: joint ----------------
            sc_j = nc.named_scope("joint"); sc_j.__enter__()
            with (
                tc.tile_pool(name="jS", bufs=1) as jS,
                tc.tile_pool(name="jPS", bufs=1, space="PSUM") as jP,
            ):
                wout_sb = jS.tile([128, JC, OD], f16, tag="woutr")
                nc.sync.dma_start(wout_sb[:], wout_d.ap())
                if with_out_bias:
                    bout_sb = jS.tile([128, OD], f32, tag="bouts")
                    nc.sync.dma_start(bout_sb[:], bout_d.ap())

                # ze (J, b*tl) and zd (J, b, u)
                sc_z = nc.named_scope("zedzd"); sc_z.__enter__()

                sc_z.__exit__(None, None, None)
                sc_b = nc.named_scope("jblk"); sc_b.__enter__()
                # joint blocks: 128 rows = 2 (b,tl) pairs x U
                n_pairs = B * TSH
                rows_per_pair = U
                ppb = 128 // rows_per_pair  # pairs per block
                BTSH = B * TSH
                for blk in range(n_pairs // ppb):
                    pr0 = blk * ppb
                    b = pr0 // TSH
                    zjt = jS.tile([128, JC, 128], f16, tag="zjt", bufs=2)
                    zj = jS.tile([128, JC, 128], f16, tag="zj", bufs=2)
                    # fused over all jc: out[p, jc, a, u] = ze[p, jc, pr0+a]
                    #                                     + zd[p, jc, u, b]
                    zjt_ap = zjt[:, :, :].rearrange(
                        "p jc (a u) -> p jc a u", a=ppb)
                    zea = ze_sb[:, 0, 0]
                    ze_bc = bass.AP(zea.tensor, zea.offset + pr0,
                                    [zea.ap[0], [BTSH, JC], [1, ppb], [0, U]])
                    zda = zd_sb[:, 0, 0, 0]
                    zd_bc = bass.AP(zda.tensor, zda.offset + b,
                                    [zda.ap[0], [U * B, JC], [0, ppb], [B, U]])
                    nc.vector.tensor_tensor(
                        zjt_ap, ze_bc, zd_bc, op=mybir.AluOpType.add)
                    nc.scalar.activation(
                        zj[:, :, :].rearrange("p jc m -> p (jc m)"),
                        zjt[:, :, :].rearrange("p jc m -> p (jc m)"), AF.Tanh)
                    ops_ = jP.tile([128, OD], f32, tag="outps", bufs=2)
                    for n2 in range(2):
                        for jc in range(JC):
                            nc.tensor.matmul(
                                ops_[:, n2 * 512:(n2 + 1) * 512],
                                zj[:, jc, :],
                                wout_sb[:, jc, n2 * 512:(n2 + 1) * 512],
                                start=(jc == 0), stop=(jc == JC - 1))
                    osb = jS.tile([128, OD], f16, tag="osb", bufs=3)
                    if with_out_bias:
                        nc.vector.tensor_add(osb[:], ops_[:], bout_sb[:])
                    else:
                        # split the psum->sbuf evacuation across DVE + ACT
                        # so neither engine bottlenecks the block pipeline
                        nc.vector.tensor_copy(osb[:, 0:512], ops_[:, 0:512])
                        nc.scalar.activation(osb[:, 512:1024],
                                             ops_[:, 512:1024], AF.Identity)
                    eng = nc.sync if blk % 2 == 0 else nc.scalar
                    eng.dma_start(
                        out_d.ap()[blk * 128:(blk + 1) * 128, :], osb[:])
                sc_b.__exit__(None, None, None)
            sc_j.__exit__(None, None, None)

    nc.compile()
    return nc


# ---------------- host-side prep ----------------

def gate_perm():
    """perm[j*1024 + s] -> row index in torch (i,f,g,o) 4H gate layout,
    with group-local order [i|f|o|g]."""
    perm = np.zeros(4 * H, dtype=np.int64)
    for j in range(NG):
        base = j * 1024
        hid = np.arange(256) + j * 256
        perm[base + 0:base + 256] = 0 * H + hid      # i
        perm[base + 256:base + 512] = 1 * H + hid    # f
        perm[base + 512:base + 768] = 3 * H + hid    # o
        perm[base + 768:base + 1024] = 2 * H + hid   # g
    return perm


def prep_inputs(hs_pad, ys_in_pad, embed, W_ih0, W_hh0, b_ih0, b_hh0,
                W_ih1, W_hh1, b_ih1, b_hh1, W_enc, b_enc, W_dec, W_out, b_out,
                U=64, n_cores=8):
    f16 = ml_dtypes.float16 if not hasattr(np, "float16") else np.float16
    perm = gate_perm()

    def wiht(W, KD, KC):  # (4H, KD) -> (128, KC, 4096) fp16, permuted gates
        Wp = W[perm]                      # (4096, KD)
        return np.ascontiguousarray(
            Wp.T.reshape(KC, 128, 4096).transpose(1, 0, 2)).astype(np.float16)

    def whht(W):  # (4H, H) -> (128, HK, NG, 1024) fp16
        Wp = W[perm]                      # (4096, 1024) rows=permuted gates
        # [p, kc, j, n] = Wp[j*1024+n, kc*128+p]
        a = Wp.T.reshape(HK, 128, NG, 1024).transpose(1, 0, 2, 3)
        return np.ascontiguousarray(a).astype(np.float16)

    ins = {}
    ins["embed"] = np.asarray(embed, np.float32)
    ys = np.asarray(ys_in_pad).astype(np.int32)   # (B, U)
    NCH = B * U // 128
    yy = np.zeros((128, NCH), np.int32)
    for ch in range(NCH):
        p = np.arange(128)
        yy[:, ch] = ys[p % 8, ch * 16 + p // 8]
    ins["yidx"] = yy
    ins["wih0t"] = wiht(W_ih0, E, EK)
    w1 = wiht(W_ih1, H, HK)  # (128, HK, 4096)
    ins["wih1t"] = np.ascontiguousarray(
        w1.reshape(128, HK, 16, 256).transpose(2, 0, 1, 3))
    ins["whh0t"] = whht(W_hh0)
    ins["whh1t"] = whht(W_hh1)
    ins["inj8"] = np.eye(8).astype(ml_dtypes.float8_e4m3)
    ins["eye128"] = np.eye(128, dtype=np.float16)
    # [p, ec, jc, m] = W[jc*128+m, ec*128+p]
    def wjt(W, KC):
        a = W.T.reshape(KC, 128, JC, 128).transpose(1, 0, 2, 3)
        return np.ascontiguousarray(a).astype(np.float16)
    ins["wenct"] = wjt(W_enc, HK)
    ins["wdect"] = wjt(W_dec, HK)
    # [p, jc, od] = W_out[od, jc*128+p]
    ins["woutt"] = np.ascontiguousarray(
        W_out.T.reshape(JC, 128, OD).transpose(1, 0, 2)).astype(np.float16)
    ins["benc"] = np.ascontiguousarray(
        b_enc.reshape(JC, 128).T).astype(np.float32)
    ins["boutrep"] = np.tile(np.asarray(b_out, np.float32)[None, :], (128, 1))
    ins["bihh0"] = np.tile(((b_ih0 + b_hh0)[perm]).astype(np.float16)[None, :],
                           (128, 1))
    ins["bihh1"] = np.tile(((b_ih1 + b_hh1)[perm]).astype(np.float16)[None, :],
                           (128, 1))

    maps = []
    for c in range(n_cores):
        m = dict(ins)
        # [p, ec, r] = hs[b, TSH*c + tl, ec*128+p], r = b*TSH+tl
        sl = np.asarray(hs_pad[:, TSH * c:TSH * (c + 1), :], np.float32)
        a = sl.reshape(B * TSH, HK, 128).transpose(2, 1, 0)
        m["hst16"] = np.ascontiguousarray(a).astype(np.float16)
        maps.append(m)
    return maps


def gather_output(results):
    outs = [np.asarray(r["out"], np.float32).reshape(B, TSH, -1, OD)
            for r in results]
    return np.concatenate(outs, axis=1)


# ---------------- entry point ----------------
import sys as _sys
import types as _types

# Recreate the missing antenv.axon_hooks so trace=True works under axon
# (used only when BASS_TRACE=1 is set by a profiling harness).
if "antenv.axon_hooks" not in _sys.modules:
    _m = _types.ModuleType("antenv.axon_hooks")

    def _get_hook():
        try:
            from trn_agent_boot.trn_boot import _ntff_profile_via_ctypes
            return _ntff_profile_via_ctypes("/opt/axon/libaxon_pjrt.so")
        except Exception:
            return None
    _m.get_axon_ntff_profile_hook = _get_hook
    _sys.modules["antenv.axon_hooks"] = _m

_NC = None
last_results = None


def kernel(**inputs):
    """Full-input RNN-T decoder: returns (B, T, U, ODIM) float32."""
    global _NC, last_results
    from concourse.bass_utils import run_bass_kernel_spmd
    U = int(np.asarray(inputs["ys_in_pad"]).shape[1])
    wb = any(float(np.abs(np.asarray(inputs[k])).max()) != 0.0
             for k in ("b_ih0", "b_hh0", "b_ih1", "b_hh1"))
    wob = float(np.abs(np.asarray(inputs["b_out"])).max()) != 0.0
    if _NC is None:
        _NC = build_program(U=U, n_cores=8, with_biases=wb, with_out_bias=wob)
    maps = prep_inputs(**inputs, U=U)
    res = run_bass_kernel_spmd(_NC, maps, core_ids=list(range(8)))
    last_results = res
    return gather_output(res.results)

